# revision 34
# baseline (speedup 1.0000x reference)
"""AdaptiveEmbedding T2I sims kernel for 8 TRN2 NeuronCores.

Sharding: caption batch 48 -> 6 per core; every core holds the full image
tensor and emits a [6, 48] slice of sims^T; host concatenates + transposes.

Math (see comments): BN folds into the FiLM affine; the softmax max-shift
cancels in the weighted mean; beta re-enters linearly. exp overflow is
prevented by capping the per-(c,d) softmax temperature s = 10*a at
+-CLAMP/X*_d where X*_d = max|x| over (i,r) for channel d — rows beyond
the cap keep their region ordering at temperature CLAMP/X* (strictly less
distortion than pointwise clamping of s*x, which flattens all clamped
regions to equal weights).

Layouts: image tiles are r-major [P, R, B] (host permutes rows to
(r, i)-major) so every fold of the region dimension is a contiguous
half-add that hits the DVE 2x bf16 mode.

Main loop per (caption c, channel block b):
  ACT   : e = exp(s'*x)            (no clamp, no intermediate)
  DVE   : p = e*x (2x), e-fold 36->18->9 (2x), strided reduce9 -> S,
          strided reduce9 of p-folds -> W
  GpSimd: p-fold 36->18->9         (keeps DVE below the ACT+fold wall)
  PE    : [capT_c | ones]^T @ [u ; u*u] accumulated over blocks -> psum
Per caption: batched fast-reciprocal of S, u assembly as 8 dual-scalar
TS ops, one uu multiply.
"""

import numpy as np
from contextlib import ExitStack

B, T, D, R = 48, 50, 1024, 36
NCORES = 8
CPC = B // NCORES          # captions per core
SMOOTH = 10.0
CLAMP = 80.0
BN_EPS = 1e-5
L2_EPS = 1e-8
P = 128
NBLK = D // P              # 8 channel blocks
NIR = B * R                # 1728 rows
NCH = (NIR + P - 1) // P   # 14 native-layout chunks

_CACHE = {}

# knobs
PFOLD_GPSIMD = True        # p-fold chain on GpSimd (else DVE)
STRIDED_RED9 = True        # strided-view reduce of f9 (else fold to end)


def _build_nc():
    import concourse.bass as bass
    import concourse.tile as tile
    from concourse import bacc, mybir
    from concourse.masks import make_identity

    FP = mybir.dt.float32
    BF = mybir.dt.bfloat16
    Alu = mybir.AluOpType
    Act = mybir.ActivationFunctionType
    Ax = mybir.AxisListType

    nc = bacc.Bacc("TRN2", target_bir_lowering=False, debug=False,
                   num_devices=NCORES)

    # imgbf rows are (r, i)-major: row r*B+i = img[i, r, :]
    imgbf = nc.dram_tensor("imgbf", (NIR, D), BF, kind="ExternalInput").ap()
    # imgT2 is the d-major copy: row d = img[:, :, d] in (r, i) order
    imgT2 = nc.dram_tensor("imgT2", (D, NIR), BF, kind="ExternalInput").ap()
    cap = nc.dram_tensor("cap", (CPC, T, D), FP, kind="ExternalInput").ap()
    maskT_d = nc.dram_tensor("maskT", (T, CPC), FP, kind="ExternalInput").ap()
    wgT_d = nc.dram_tensor("wgT", (D, D), FP, kind="ExternalInput").ap()
    wbT_d = nc.dram_tensor("wbT", (D, D), FP, kind="ExternalInput").ap()
    bg1T_d = nc.dram_tensor("bg1T", (P, NBLK), FP, kind="ExternalInput").ap()
    bbT_d = nc.dram_tensor("bbT", (P, NBLK), FP, kind="ExternalInput").ap()
    out_d = nc.dram_tensor("out", (CPC, B), FP, kind="ExternalOutput").ap()

    with tile.TileContext(nc) as tc, ExitStack() as ctx:
        consts = ctx.enter_context(tc.tile_pool(name="consts", bufs=1))
        ident = consts.tile([P, P], FP, tag="ident")
        make_identity(nc, ident[:])
        ones1b = consts.tile([P, 1], BF, tag="ones1b")
        nc.vector.memset(ones1b[:], 1.0)

        smalls = ctx.enter_context(tc.tile_pool(name="smalls", bufs=1))
        cap_pool = ctx.enter_context(tc.tile_pool(name="cap", bufs=3))
        tp_psum = ctx.enter_context(tc.tile_pool(name="tp_ps", bufs=2,
                                                 space="PSUM"))
        sq_pool = ctx.enter_context(tc.tile_pool(name="sq", bufs=2))
        w_pool = ctx.enter_context(tc.tile_pool(name="w", bufs=3))
        gcd_pool = ctx.enter_context(tc.tile_pool(name="gcd", bufs=2))
        xall_pool = ctx.enter_context(tc.tile_pool(name="xall", bufs=1))

        # ---------- caption branch ----------
        maskT = smalls.tile([T, CPC], FP, tag="maskT")
        nc.sync.dma_start(out=maskT[:], in_=maskT_d[:, :])
        cap_sb = smalls.tile([CPC, D], FP, tag="cap_sb")
        with tc.tile_pool(name="cap_ps", bufs=2, space="PSUM") as cap_ps_pool:
            for c in range(CPC):
                ct = cap_pool.tile([T, D], FP, tag="cap")
                nc.sync.dma_start(out=ct[:], in_=cap[c, :, :])
                pp = cap_ps_pool.tile([1, D], FP, tag="pp", name="pp")
                for j in range(2):
                    nc.tensor.matmul(pp[:, 512 * j:512 * (j + 1)],
                                     maskT[:, c:c + 1],
                                     ct[:, 512 * j:512 * (j + 1)],
                                     start=True, stop=True,
                                     skip_group_check=True)
                prow = cap_pool.tile([1, D], FP, tag="prow", name="prow",
                                     bufs=2)
                nc.scalar.copy(prow[:], pp[:])
                nc.sync.dma_start(out=cap_sb[c:c + 1, :], in_=prow[:])

        # weight half-0 loads issued early (before the transpose-DMA flood);
        # pool closed manually right after the FiLM matmuls free the space
        w0_cm = tc.tile_pool(name="w0", bufs=1)
        w0_pool = w0_cm.__enter__()
        w0 = {}
        for which, wd in (("g", wgT_d), ("b", wbT_d)):
            for kb in range(NBLK):
                t = w0_pool.tile([P, D // 2], FP, tag=f"w0{which}{kb}",
                                 name=f"w0{which}{kb}")
                nc.sync.dma_start(out=t[:], in_=wd[P * kb:P * (kb + 1), 0:512])
                w0[(which, kb)] = t

        capT = [smalls.tile([P, CPC], FP, tag=f"capT{b}", name=f"capT{b}")
                for b in range(NBLK)]
        capT2 = [smalls.tile([P, CPC, 2], BF, tag=f"capT2{b}",
                             name=f"capT2{b}") for b in range(NBLK)]
        for blk in range(NBLK):
            nc.vector.memset(capT2[blk][:], 1.0)
            pst = tp_psum.tile([P, P], FP, tag="tp")
            nc.tensor.transpose(pst[:, 0:CPC], cap_sb[:, P * blk:P * (blk + 1)],
                                ident[:CPC, :CPC])
            nc.vector.tensor_copy(out=capT[blk][:], in_=pst[:, 0:CPC])
            nc.vector.tensor_copy(out=capT2[blk][:, :, 0], in_=pst[:, 0:CPC])

        scr_c = smalls.tile([CPC, D], FP, tag="scr_c")
        nc.vector.tensor_tensor(out=scr_c[:], in0=cap_sb[:], in1=cap_sb[:],
                                op=Alu.mult)
        n2 = smalls.tile([CPC, 1], FP, tag="n2")
        nc.vector.tensor_reduce(out=n2[:], in_=scr_c[:], axis=Ax.X, op=Alu.add)
        nrm = smalls.tile([CPC, 1], FP, tag="nrm")
        nc.scalar.activation(nrm[:], n2[:], Act.Sqrt)
        nrm_e = smalls.tile([CPC, 1], FP, tag="nrm_e")
        nc.vector.tensor_scalar(out=nrm_e[:], in0=nrm[:], scalar1=L2_EPS,
                                scalar2=None, op0=Alu.add)
        rn = smalls.tile([CPC, 1], FP, tag="rn")
        nc.vector.reciprocal(rn[:], nrm_e[:])

        # ---------- image loads ----------
        # both layouts come pre-transposed from the host: plain row DMAs only
        xall = [xall_pool.tile([P, R, B], BF, tag=f"xall{b}", name=f"xall{b}")
                for b in range(NBLK)]
        for blk in range(NBLK):
            nc.sync.dma_start(
                out=xall[blk][:].rearrange("p r i -> p (r i)"),
                in_=imgT2[P * blk:P * (blk + 1), :])
        sums_sb = smalls.tile([1, D], FP, tag="sums_sb")
        muT = smalls.tile([P, NBLK], FP, tag="muT")
        with tc.tile_pool(name="xt", bufs=1) as xt_pool, \
             tc.tile_pool(name="xs_ps", bufs=1, space="PSUM") as xs_ps_pool:
            xt = [xt_pool.tile([P, D], BF, tag=f"xt{k}", name=f"xt{k}")
                  for k in range(NCH)]
            for k in range(NCH):
                rows = min(P, NIR - P * k)
                nc.sync.dma_start(out=xt[k][0:rows, :],
                                  in_=imgbf[P * k:P * k + rows, :])
            ps = [xs_ps_pool.tile([1, 512], FP, tag=f"xs{h}", name=f"xs{h}")
                  for h in range(2)]
            for h in range(2):
                for k in range(NCH):
                    rows = min(P, NIR - P * k)
                    nc.tensor.matmul(ps[h][:], ones1b[0:rows, :],
                                     xt[k][0:rows, 512 * h:512 * (h + 1)],
                                     start=(k == 0), stop=(k == NCH - 1),
                                     skip_group_check=True)
            for h in range(2):
                nc.scalar.copy(sums_sb[:, 512 * h:512 * (h + 1)], ps[h][:])
        inv_n = 1.0 / float(NIR)
        muT_raw = smalls.tile([P, NBLK], FP, tag="muT_raw")
        for b2 in range(NBLK):
            pst = tp_psum.tile([P, P], FP, tag="tp")
            nc.tensor.transpose(pst[:, 0:1],
                                sums_sb[:, P * b2:P * (b2 + 1)],
                                ident[:1, :1])
            nc.vector.tensor_copy(out=muT_raw[:, b2:b2 + 1], in_=pst[:, 0:1])
        nc.vector.tensor_scalar(out=muT[:], in0=muT_raw[:], scalar1=inv_n,
                                scalar2=None, op0=Alu.mult)

        # x^2 sums (DVE contiguous folds) and X* = max|x| (GpSimd folds)
        m2T = smalls.tile([P, NBLK], FP, tag="m2T")
        xstar = smalls.tile([P, NBLK], FP, tag="xstar")
        for blk in range(NBLK):
            xf = xall[blk][:].rearrange("p r i -> p (r i)")
            x2 = sq_pool.tile([P, NIR], BF, tag="x2")
            nc.scalar.square(x2[:], xf)
            f1 = sq_pool.tile([P, NIR // 2], BF, tag="f1")
            nc.vector.tensor_tensor(out=f1[:], in0=x2[:, 0:NIR // 2],
                                    in1=x2[:, NIR // 2:NIR], op=Alu.add)
            f2 = sq_pool.tile([P, NIR // 4], BF, tag="f2")
            nc.vector.tensor_tensor(out=f2[:], in0=f1[:, 0:NIR // 4],
                                    in1=f1[:, NIR // 4:NIR // 2], op=Alu.add)
            sq_s = sq_pool.tile([P, 1], FP, tag="sq_s", bufs=2)
            nc.vector.tensor_reduce(out=sq_s[:], in_=f2[:], axis=Ax.X,
                                    op=Alu.add)
            nc.vector.tensor_scalar(out=m2T[:, blk:blk + 1], in0=sq_s[:],
                                    scalar1=inv_n, scalar2=None, op0=Alu.mult)
            # X* = max|x| per channel, one absolute-value max-reduce
            nc.vector.tensor_reduce(out=xstar[:, blk:blk + 1], in_=xf,
                                    axis=Ax.X, op=Alu.max,
                                    apply_absolute_value=True)

        # per-channel temperature cap thrS = CLAMP / X*
        rxs = smalls.tile([P, NBLK], FP, tag="rxs")
        nc.vector.reciprocal_approx_fast(out=rxs[:], in_=xstar[:])
        thrS = smalls.tile([P, NBLK], FP, tag="thrS")
        nc.vector.tensor_scalar(out=thrS[:], in0=rxs[:], scalar1=CLAMP,
                                scalar2=None, op0=Alu.mult)
        nthrS = smalls.tile([P, NBLK], FP, tag="nthrS")
        nc.vector.tensor_scalar(out=nthrS[:], in0=thrS[:], scalar1=-1.0,
                                scalar2=None, op0=Alu.mult)

        # BN: rho10 = 10/sqrt(var+eps), mu01 = mu/10
        musqT = smalls.tile([P, NBLK], FP, tag="musqT")
        nc.scalar.square(musqT[:], muT[:])
        varT = smalls.tile([P, NBLK], FP, tag="varT")
        nc.vector.tensor_tensor(out=varT[:], in0=m2T[:], in1=musqT[:],
                                op=Alu.subtract)
        epsT = smalls.tile([P, 1], FP, tag="epsT")
        nc.vector.memset(epsT[:], BN_EPS)
        stdT = smalls.tile([P, NBLK], FP, tag="stdT")
        nc.scalar.activation(stdT[:], varT[:], Act.Sqrt, bias=epsT[:])
        rhoT = smalls.tile([P, NBLK], FP, tag="rhoT")
        nc.vector.reciprocal_approx_fast(out=rhoT[:], in_=stdT[:])
        rho10 = smalls.tile([P, NBLK], FP, tag="rho10")
        nc.vector.tensor_scalar(out=rho10[:], in0=rhoT[:], scalar1=SMOOTH,
                                scalar2=None, op0=Alu.mult)
        mu01 = smalls.tile([P, NBLK], FP, tag="mu01")
        nc.vector.tensor_scalar(out=mu01[:], in0=muT[:], scalar1=1.0 / SMOOTH,
                                scalar2=None, op0=Alu.mult)

        # ---------- FiLM params ----------
        bg1T = smalls.tile([P, NBLK], FP, tag="bg1T")
        nc.sync.dma_start(out=bg1T[:], in_=bg1T_d[:, :])
        bbT = smalls.tile([P, NBLK], FP, tag="bbT")
        nc.sync.dma_start(out=bbT[:], in_=bbT_d[:, :])

        g_all = smalls.tile([P, NBLK, CPC], FP, tag="g_all")
        bta_all = smalls.tile([P, NBLK, CPC], FP, tag="bta_all")
        with tc.tile_pool(name="gb_ps", bufs=4, space="PSUM") as gb_ps_pool:
            for which, wd, dst in (("g", wgT_d, g_all), ("b", wbT_d, bta_all)):
                for half in range(2):
                    ps = gb_ps_pool.tile([CPC, 512], FP, tag="gcd",
                                         name="gcd_ps")
                    for kb in range(NBLK):
                        if half == 0:
                            w = w0[(which, kb)]
                        else:
                            w = w_pool.tile([P, D // 2], FP, tag="w", name="w")
                            nc.sync.dma_start(
                                out=w[:], in_=wd[P * kb:P * (kb + 1),
                                                 512 * half:512 * (half + 1)])
                        nc.tensor.matmul(ps[:], capT[kb][:], w[:],
                                         start=(kb == 0),
                                         stop=(kb == NBLK - 1),
                                         skip_group_check=True)
                    gsb = gcd_pool.tile([CPC, 512], FP, tag="gsb", name="gsb")
                    nc.scalar.copy(gsb[:], ps[:])
                    for j in range(4):
                        db = half * 4 + j
                        pst = tp_psum.tile([P, P], FP, tag="tp")
                        nc.tensor.transpose(pst[:, 0:CPC],
                                            gsb[:, P * j:P * (j + 1)],
                                            ident[:CPC, :CPC])
                        nc.vector.tensor_copy(out=dst[:, db, :],
                                              in_=pst[:, 0:CPC])
        w0_cm.__exit__(None, None, None)

        def bcast_col(t2d):
            return t2d[:].rearrange("p b -> p b ()").broadcast_to(
                [P, NBLK, CPC])

        # a10 = (g + bg1)*rho10 (the raw temperature), aT = a10/10
        a10 = smalls.tile([P, NBLK, CPC], FP, tag="a10")
        t_a = smalls.tile([P, NBLK, CPC], FP, tag="t_a")
        nc.vector.tensor_tensor(out=t_a[:], in0=g_all[:], in1=bcast_col(bg1T),
                                op=Alu.add)
        nc.vector.tensor_tensor(out=a10[:], in0=t_a[:], in1=bcast_col(rho10),
                                op=Alu.mult)
        aT = smalls.tile([P, NBLK, CPC], FP, tag="aT")
        nc.vector.tensor_scalar(out=aT[:], in0=a10[:], scalar1=1.0 / SMOOTH,
                                scalar2=None, op0=Alu.mult)
        # capped temperature s' (used only as the exp scale)
        sc_all = smalls.tile([P, NBLK, CPC], FP, tag="sc_all")
        for blk in range(NBLK):
            nc.vector.tensor_scalar(out=sc_all[:, blk, :],
                                    in0=a10[:, blk, :],
                                    scalar1=thrS[:, blk:blk + 1],
                                    scalar2=nthrS[:, blk:blk + 1],
                                    op0=Alu.min, op1=Alu.max)
        # negb2 = a10*mu01 - (bta + bb)   (u = wa - negb2)
        negb2 = smalls.tile([P, NBLK, CPC], FP, tag="negb2")
        t_b = smalls.tile([P, NBLK, CPC], FP, tag="t_b")
        nc.vector.tensor_tensor(out=t_b[:], in0=bta_all[:],
                                in1=bcast_col(bbT), op=Alu.add)
        t_c2 = smalls.tile([P, NBLK, CPC], FP, tag="t_c2")
        nc.vector.tensor_tensor(out=t_c2[:], in0=a10[:], in1=bcast_col(mu01),
                                op=Alu.mult)
        nc.vector.tensor_tensor(out=negb2[:], in0=t_c2[:], in1=t_b[:],
                                op=Alu.subtract)
        # pre-expanded (over images) copies for the contiguous u-assembly
        aT_exp = smalls.tile([P, NBLK, CPC, B], BF, tag="aT_exp")
        nc.vector.tensor_copy(
            out=aT_exp[:],
            in_=aT[:].rearrange("p b c -> p b c ()").broadcast_to(
                [P, NBLK, CPC, B]))
        nb2_exp = smalls.tile([P, NBLK, CPC, B], BF, tag="nb2_exp")
        nc.vector.tensor_copy(
            out=nb2_exp[:],
            in_=negb2[:].rearrange("p b c -> p b c ()").broadcast_to(
                [P, NBLK, CPC, B]))

        # ---------- main loop ----------
        big_pool = ctx.enter_context(tc.tile_pool(name="big", bufs=3))
        sw_pool = ctx.enter_context(tc.tile_pool(name="sw", bufs=2))
        dots_sb = smalls.tile([CPC, B], FP, tag="dots_sb")
        usq_sb = smalls.tile([CPC, B], FP, tag="usq_sb")
        with tc.tile_pool(name="du_ps", bufs=3, space="PSUM") as du_ps_pool:
            for c in range(CPC):
                F9e = sw_pool.tile([P, NBLK, 9, B], BF, tag="F9e")
                F9p = sw_pool.tile([P, NBLK, 9, B], BF, tag="F9p")
                ps_du = du_ps_pool.tile([2, 2 * B], FP, tag="du")
                for blk in range(NBLK):
                    e = big_pool.tile([P, R, B], BF, tag="e", bufs=3)
                    nc.scalar.activation(e[:], xall[blk][:], Act.Exp,
                                         scale=sc_all[:, blk, c:c + 1])
                    p = big_pool.tile([P, R, B], BF, tag="p", bufs=3)
                    nc.vector.tensor_tensor(out=p[:], in0=e[:],
                                            in1=xall[blk][:], op=Alu.mult)
                    f18e = big_pool.tile([P, 18, B], BF, tag="f18e", bufs=3)
                    nc.vector.tensor_tensor(out=f18e[:], in0=e[:, 0:18, :],
                                            in1=e[:, 18:36, :], op=Alu.add)
                    nc.vector.tensor_tensor(out=F9e[:, blk],
                                            in0=f18e[:, 0:9, :],
                                            in1=f18e[:, 9:18, :], op=Alu.add)
                    geng = nc.gpsimd if PFOLD_GPSIMD else nc.vector
                    f18p = big_pool.tile([P, 18, B], BF, tag="f18p", bufs=3)
                    geng.tensor_tensor(out=f18p[:], in0=p[:, 0:18, :],
                                       in1=p[:, 18:36, :], op=Alu.add)
                    geng.tensor_tensor(out=F9p[:, blk],
                                       in0=f18p[:, 0:9, :],
                                       in1=f18p[:, 9:18, :], op=Alu.add)
                # batched fold tails: 9 -> (4+4+1) -> 2 -> 1, all blocks at
                # once, folding in place inside the F9 accumulators
                Sall = sw_pool.tile([P, NBLK, B], FP, tag="Sall")
                Wall = sw_pool.tile([P, NBLK, B], FP, tag="Wall")
                for F9, dst in ((F9e, Sall), (F9p, Wall)):
                    nc.vector.tensor_tensor(out=F9[:, :, 0:4, :],
                                            in0=F9[:, :, 0:4, :],
                                            in1=F9[:, :, 4:8, :], op=Alu.add)
                    nc.vector.tensor_tensor(out=F9[:, :, 0:2, :],
                                            in0=F9[:, :, 0:2, :],
                                            in1=F9[:, :, 2:4, :], op=Alu.add)
                    nc.vector.tensor_tensor(out=F9[:, :, 0, :],
                                            in0=F9[:, :, 0, :],
                                            in1=F9[:, :, 1, :], op=Alu.add)
                    nc.vector.tensor_tensor(out=dst[:], in0=F9[:, :, 0, :],
                                            in1=F9[:, :, 8, :], op=Alu.add)
                rs = sw_pool.tile([P, NBLK, B], FP, tag="rs")
                nc.vector.reciprocal_approx_fast(
                    out=rs[:].rearrange("p b i -> p (b i)"),
                    in_=Sall[:].rearrange("p b i -> p (b i)"))
                t1 = sw_pool.tile([P, NBLK, B], BF, tag="t1")
                nc.vector.tensor_tensor(out=t1[:], in0=Wall[:], in1=rs[:],
                                        op=Alu.mult)
                uu_big = sw_pool.tile([P, 2, NBLK, B], BF, tag="uu")
                t2 = sw_pool.tile([P, NBLK, B], BF, tag="t2")
                nc.vector.tensor_tensor(out=t2[:], in0=t1[:],
                                        in1=aT_exp[:, :, c, :], op=Alu.mult)
                nc.vector.tensor_tensor(out=uu_big[:, 0], in0=t2[:],
                                        in1=nb2_exp[:, :, c, :],
                                        op=Alu.subtract)
                nc.vector.tensor_tensor(out=uu_big[:, 1], in0=uu_big[:, 0],
                                        in1=uu_big[:, 0], op=Alu.mult)
                for blk in range(NBLK):
                    nc.tensor.matmul(ps_du[:], capT2[blk][:, c, :],
                                     uu_big[:, :, blk, :],
                                     start=(blk == 0), stop=(blk == NBLK - 1),
                                     skip_group_check=True)
                du_c = sw_pool.tile([2, 2 * B], FP, tag="du_c", bufs=2)
                nc.scalar.copy(du_c[:], ps_du[:])
                nc.sync.dma_start(out=dots_sb[c:c + 1, :],
                                  in_=du_c[0:1, 0:B])
                nc.sync.dma_start(out=usq_sb[c:c + 1, :],
                                  in_=du_c[1:2, B:2 * B])

        # ---------- tail ----------
        sq = smalls.tile([CPC, B], FP, tag="sqf")
        nc.scalar.activation(sq[:], usq_sb[:], Act.Sqrt)
        ru = smalls.tile([CPC, B], FP, tag="ruf")
        nc.vector.reciprocal_approx_fast(out=ru[:], in_=sq[:])
        t3 = smalls.tile([CPC, B], FP, tag="t3f")
        nc.vector.tensor_tensor(out=t3[:], in0=dots_sb[:], in1=ru[:],
                                op=Alu.mult)
        out_sb = smalls.tile([CPC, B], FP, tag="out_sb")
        nc.vector.tensor_scalar(out=out_sb[:], in0=t3[:], scalar1=rn[:, 0:1],
                                scalar2=None, op0=Alu.mult)
        nc.sync.dma_start(out=out_d[:, :], in_=out_sb[:])

    nc.compile()
    return nc


def _get_nc():
    if "nc" not in _CACHE:
        _CACHE["nc"] = _build_nc()
    return _CACHE["nc"]


def kernel(img_embed, cap_embed, lens, W_gamma, b_gamma, W_beta, b_beta,
           _want_trace=False):
    from concourse.bass_utils import run_bass_kernel_spmd

    nc = _get_nc()

    img_embed = np.asarray(img_embed, np.float32)
    cap_embed = np.asarray(cap_embed, np.float32)
    lens_np = np.asarray(lens)
    W_gamma = np.asarray(W_gamma, np.float32)
    W_beta = np.asarray(W_beta, np.float32)
    b_gamma = np.asarray(b_gamma, np.float32)
    b_beta = np.asarray(b_beta, np.float32)

    import ml_dtypes
    # rows (r, i)-major so the on-chip tiles are [P, R, B]
    img_ri = img_embed.transpose(1, 0, 2).reshape(NIR, D)
    img_bf = np.ascontiguousarray(img_ri.astype(ml_dtypes.bfloat16))
    img_t2 = np.ascontiguousarray(img_bf.T)
    wgT = np.ascontiguousarray(W_gamma.T)
    wbT = np.ascontiguousarray(W_beta.T)
    bg1T = np.ascontiguousarray((1.0 + b_gamma).reshape(NBLK, P).T)
    bbT = np.ascontiguousarray(b_beta.reshape(NBLK, P).T)

    lens_f = lens_np.astype(np.float64)
    mask = (np.arange(T)[None, :] < lens_np[:, None]).astype(np.float64)
    mask = (mask / lens_f[:, None]).astype(np.float32)  # (B, T)

    in_maps = []
    for k in range(NCORES):
        sl = slice(k * CPC, (k + 1) * CPC)
        in_maps.append({
            "imgbf": img_bf,
            "imgT2": img_t2,
            "cap": np.ascontiguousarray(cap_embed[sl]),
            "maskT": np.ascontiguousarray(mask[sl].T),
            "wgT": wgT,
            "wbT": wbT,
            "bg1T": bg1T,
            "bbT": bbT,
        })

    kw = {}
    if _want_trace:
        import os as _os2, shutil as _sh
        _sh.rmtree("/tmp/ktrace", ignore_errors=True)
        _os2.makedirs("/tmp/ktrace", exist_ok=True)
        kw = {"tmpdir": "/tmp/ktrace"}
    res = run_bass_kernel_spmd(nc, in_maps, core_ids=list(range(NCORES)),
                               trace=_want_trace, **kw)
    outs = [np.asarray(r["out"]) for r in res.results]
    sims = np.concatenate([o.T for o in outs], axis=1).astype(np.float32)
    if _want_trace:
        return sims, res
    return sims


# revision 35
# speedup vs baseline: 1.2090x; 1.2090x over previous
"""AdaptiveEmbedding T2I sims kernel for 8 TRN2 NeuronCores.

Sharding: caption batch 48 -> 6 per core; every core holds the full image
tensor and emits a [6, 48] slice of sims^T; host concatenates + transposes.

Math (see comments): BN folds into the FiLM affine; the softmax max-shift
cancels in the weighted mean; beta re-enters linearly. exp overflow is
prevented by capping the per-(c,d) softmax temperature s = 10*a at
+-CLAMP/X*_d where X*_d = max|x| over (i,r) for channel d — rows beyond
the cap keep their region ordering at temperature CLAMP/X* (strictly less
distortion than pointwise clamping of s*x, which flattens all clamped
regions to equal weights).

Layouts: image tiles are r-major [P, R, B] (host permutes rows to
(r, i)-major) so every fold of the region dimension is a contiguous
half-add that hits the DVE 2x bf16 mode.

Main loop per (caption c, channel block b):
  ACT   : e = exp(s'*x)            (no clamp, no intermediate)
  DVE   : p = e*x (2x), e-fold 36->18->9 (2x), strided reduce9 -> S,
          strided reduce9 of p-folds -> W
  GpSimd: p-fold 36->18->9         (keeps DVE below the ACT+fold wall)
  PE    : [capT_c | ones]^T @ [u ; u*u] accumulated over blocks -> psum
Per caption: batched fast-reciprocal of S, u assembly as 8 dual-scalar
TS ops, one uu multiply.
"""

import numpy as np
from contextlib import ExitStack

B, T, D, R = 48, 50, 1024, 36
NCORES = 8
CPC = B // NCORES          # captions per core
SMOOTH = 10.0
CLAMP = 80.0
BN_EPS = 1e-5
L2_EPS = 1e-8
P = 128
NBLK = D // P              # 8 channel blocks
NIR = B * R                # 1728 rows
NCH = (NIR + P - 1) // P   # 14 native-layout chunks

_CACHE = {}

# knobs
PFOLD_GPSIMD = True        # p-fold chain on GpSimd (else DVE)
STRIDED_RED9 = True        # strided-view reduce of f9 (else fold to end)


def _build_nc():
    import concourse.bass as bass
    import concourse.tile as tile
    from concourse import bacc, mybir
    from concourse.masks import make_identity

    FP = mybir.dt.float32
    BF = mybir.dt.bfloat16
    Alu = mybir.AluOpType
    Act = mybir.ActivationFunctionType
    Ax = mybir.AxisListType

    nc = bacc.Bacc("TRN2", target_bir_lowering=False, debug=False,
                   num_devices=NCORES)

    # imgbf rows are (r, i)-major: row r*B+i = img[i, r, :]
    imgbf = nc.dram_tensor("imgbf", (NIR, D), BF, kind="ExternalInput").ap()
    # imgT2 is the d-major copy: row d = img[:, :, d] in (r, i) order
    imgT2 = nc.dram_tensor("imgT2", (D, NIR), BF, kind="ExternalInput").ap()
    cap = nc.dram_tensor("cap", (CPC, T, D), FP, kind="ExternalInput").ap()
    maskT_d = nc.dram_tensor("maskT", (T, CPC), FP, kind="ExternalInput").ap()
    wgT_d = nc.dram_tensor("wgT", (D, D), FP, kind="ExternalInput").ap()
    wbT_d = nc.dram_tensor("wbT", (D, D), FP, kind="ExternalInput").ap()
    bg1T_d = nc.dram_tensor("bg1T", (P, NBLK), FP, kind="ExternalInput").ap()
    bbT_d = nc.dram_tensor("bbT", (P, NBLK), FP, kind="ExternalInput").ap()
    out_d = nc.dram_tensor("out", (CPC, B), FP, kind="ExternalOutput").ap()

    with tile.TileContext(nc) as tc, ExitStack() as ctx:
        consts = ctx.enter_context(tc.tile_pool(name="consts", bufs=1))
        ident = consts.tile([P, P], FP, tag="ident")
        make_identity(nc, ident[:])
        ones1b = consts.tile([P, 1], BF, tag="ones1b")
        nc.vector.memset(ones1b[:], 1.0)

        smalls = ctx.enter_context(tc.tile_pool(name="smalls", bufs=1))
        cap_pool = ctx.enter_context(tc.tile_pool(name="cap", bufs=3))
        tp_psum = ctx.enter_context(tc.tile_pool(name="tp_ps", bufs=2,
                                                 space="PSUM"))
        sq_pool = ctx.enter_context(tc.tile_pool(name="sq", bufs=2))
        w_pool = ctx.enter_context(tc.tile_pool(name="w", bufs=3))
        gcd_pool = ctx.enter_context(tc.tile_pool(name="gcd", bufs=2))
        xall_pool = ctx.enter_context(tc.tile_pool(name="xall", bufs=1))

        # image tiles first — they gate X*/stats and the main loop;
        # both layouts come pre-transposed from the host: plain row DMAs only
        xall = [xall_pool.tile([P, R, B], BF, tag=f"xall{b}", name=f"xall{b}")
                for b in range(NBLK)]
        for blk in range(NBLK):
            nc.sync.dma_start(
                out=xall[blk][:].rearrange("p r i -> p (r i)"),
                in_=imgT2[P * blk:P * (blk + 1), :])

        # ---------- caption branch ----------
        maskT = smalls.tile([T, CPC], FP, tag="maskT")
        nc.sync.dma_start(out=maskT[:], in_=maskT_d[:, :])
        cap_sb = smalls.tile([CPC, D], FP, tag="cap_sb")
        with tc.tile_pool(name="cap_ps", bufs=2, space="PSUM") as cap_ps_pool:
            for c in range(CPC):
                ct = cap_pool.tile([T, D], FP, tag="cap")
                nc.sync.dma_start(out=ct[:], in_=cap[c, :, :])
                pp = cap_ps_pool.tile([1, D], FP, tag="pp", name="pp")
                for j in range(2):
                    nc.tensor.matmul(pp[:, 512 * j:512 * (j + 1)],
                                     maskT[:, c:c + 1],
                                     ct[:, 512 * j:512 * (j + 1)],
                                     start=True, stop=True,
                                     skip_group_check=True)
                prow = cap_pool.tile([1, D], FP, tag="prow", name="prow",
                                     bufs=2)
                nc.scalar.copy(prow[:], pp[:])
                nc.sync.dma_start(out=cap_sb[c:c + 1, :], in_=prow[:])

        # weight half-0 loads issued early (before the transpose-DMA flood);
        # pool closed manually right after the FiLM matmuls free the space
        w0_cm = tc.tile_pool(name="w0", bufs=1)
        w0_pool = w0_cm.__enter__()
        w0 = {}
        for which, wd in (("g", wgT_d), ("b", wbT_d)):
            for kb in range(NBLK):
                t = w0_pool.tile([P, D // 2], FP, tag=f"w0{which}{kb}",
                                 name=f"w0{which}{kb}")
                nc.sync.dma_start(out=t[:], in_=wd[P * kb:P * (kb + 1), 0:512])
                w0[(which, kb)] = t

        capT = [smalls.tile([P, CPC], FP, tag=f"capT{b}", name=f"capT{b}")
                for b in range(NBLK)]
        capT2 = [smalls.tile([P, CPC, 2], BF, tag=f"capT2{b}",
                             name=f"capT2{b}") for b in range(NBLK)]
        for blk in range(NBLK):
            nc.vector.memset(capT2[blk][:], 1.0)
            pst = tp_psum.tile([P, P], FP, tag="tp")
            nc.tensor.transpose(pst[:, 0:CPC], cap_sb[:, P * blk:P * (blk + 1)],
                                ident[:CPC, :CPC])
            nc.vector.tensor_copy(out=capT[blk][:], in_=pst[:, 0:CPC])
            nc.vector.tensor_copy(out=capT2[blk][:, :, 0], in_=pst[:, 0:CPC])

        scr_c = smalls.tile([CPC, D], FP, tag="scr_c")
        nc.vector.tensor_tensor(out=scr_c[:], in0=cap_sb[:], in1=cap_sb[:],
                                op=Alu.mult)
        n2 = smalls.tile([CPC, 1], FP, tag="n2")
        nc.vector.tensor_reduce(out=n2[:], in_=scr_c[:], axis=Ax.X, op=Alu.add)
        nrm = smalls.tile([CPC, 1], FP, tag="nrm")
        nc.scalar.activation(nrm[:], n2[:], Act.Sqrt)
        nrm_e = smalls.tile([CPC, 1], FP, tag="nrm_e")
        nc.vector.tensor_scalar(out=nrm_e[:], in0=nrm[:], scalar1=L2_EPS,
                                scalar2=None, op0=Alu.add)
        rn = smalls.tile([CPC, 1], FP, tag="rn")
        nc.vector.reciprocal(rn[:], nrm_e[:])

        # ---------- image loads ----------
        sums_sb = smalls.tile([1, D], FP, tag="sums_sb")
        muT = smalls.tile([P, NBLK], FP, tag="muT")
        with tc.tile_pool(name="xt", bufs=1) as xt_pool, \
             tc.tile_pool(name="xs_ps", bufs=1, space="PSUM") as xs_ps_pool:
            xt = [xt_pool.tile([P, D], BF, tag=f"xt{k}", name=f"xt{k}")
                  for k in range(NCH)]
            for k in range(NCH):
                rows = min(P, NIR - P * k)
                nc.sync.dma_start(out=xt[k][0:rows, :],
                                  in_=imgbf[P * k:P * k + rows, :])
            ps = [xs_ps_pool.tile([1, 512], FP, tag=f"xs{h}", name=f"xs{h}")
                  for h in range(2)]
            for h in range(2):
                for k in range(NCH):
                    rows = min(P, NIR - P * k)
                    nc.tensor.matmul(ps[h][:], ones1b[0:rows, :],
                                     xt[k][0:rows, 512 * h:512 * (h + 1)],
                                     start=(k == 0), stop=(k == NCH - 1),
                                     skip_group_check=True)
            for h in range(2):
                nc.scalar.copy(sums_sb[:, 512 * h:512 * (h + 1)], ps[h][:])
        inv_n = 1.0 / float(NIR)
        muT_raw = smalls.tile([P, NBLK], FP, tag="muT_raw")
        for b2 in range(NBLK):
            pst = tp_psum.tile([P, P], FP, tag="tp")
            nc.tensor.transpose(pst[:, 0:1],
                                sums_sb[:, P * b2:P * (b2 + 1)],
                                ident[:1, :1])
            nc.vector.tensor_copy(out=muT_raw[:, b2:b2 + 1], in_=pst[:, 0:1])
        nc.vector.tensor_scalar(out=muT[:], in0=muT_raw[:], scalar1=inv_n,
                                scalar2=None, op0=Alu.mult)

        # x^2 sums (DVE contiguous folds) and X* = max|x| (GpSimd folds)
        m2T = smalls.tile([P, NBLK], FP, tag="m2T")
        xstar = smalls.tile([P, NBLK], FP, tag="xstar")
        for blk in range(NBLK):
            xf = xall[blk][:].rearrange("p r i -> p (r i)")
            x2 = sq_pool.tile([P, NIR], BF, tag="x2")
            nc.scalar.square(x2[:], xf)
            f1 = sq_pool.tile([P, NIR // 2], BF, tag="f1")
            nc.vector.tensor_tensor(out=f1[:], in0=x2[:, 0:NIR // 2],
                                    in1=x2[:, NIR // 2:NIR], op=Alu.add)
            f2 = sq_pool.tile([P, NIR // 4], BF, tag="f2")
            nc.vector.tensor_tensor(out=f2[:], in0=f1[:, 0:NIR // 4],
                                    in1=f1[:, NIR // 4:NIR // 2], op=Alu.add)
            sq_s = sq_pool.tile([P, 1], FP, tag="sq_s", bufs=2)
            nc.vector.tensor_reduce(out=sq_s[:], in_=f2[:], axis=Ax.X,
                                    op=Alu.add)
            nc.vector.tensor_scalar(out=m2T[:, blk:blk + 1], in0=sq_s[:],
                                    scalar1=inv_n, scalar2=None, op0=Alu.mult)
            # X* = max|x| per channel, one absolute-value max-reduce
            nc.vector.tensor_reduce(out=xstar[:, blk:blk + 1], in_=xf,
                                    axis=Ax.X, op=Alu.max,
                                    apply_absolute_value=True)

        # per-channel temperature cap thrS = CLAMP / X*
        rxs = smalls.tile([P, NBLK], FP, tag="rxs")
        nc.vector.reciprocal_approx_fast(out=rxs[:], in_=xstar[:])
        thrS = smalls.tile([P, NBLK], FP, tag="thrS")
        nc.vector.tensor_scalar(out=thrS[:], in0=rxs[:], scalar1=CLAMP,
                                scalar2=None, op0=Alu.mult)
        nthrS = smalls.tile([P, NBLK], FP, tag="nthrS")
        nc.vector.tensor_scalar(out=nthrS[:], in0=thrS[:], scalar1=-1.0,
                                scalar2=None, op0=Alu.mult)

        # BN: rho10 = 10/sqrt(var+eps), mu01 = mu/10
        musqT = smalls.tile([P, NBLK], FP, tag="musqT")
        nc.scalar.square(musqT[:], muT[:])
        varT = smalls.tile([P, NBLK], FP, tag="varT")
        nc.vector.tensor_tensor(out=varT[:], in0=m2T[:], in1=musqT[:],
                                op=Alu.subtract)
        epsT = smalls.tile([P, 1], FP, tag="epsT")
        nc.vector.memset(epsT[:], BN_EPS)
        stdT = smalls.tile([P, NBLK], FP, tag="stdT")
        nc.scalar.activation(stdT[:], varT[:], Act.Sqrt, bias=epsT[:])
        rhoT = smalls.tile([P, NBLK], FP, tag="rhoT")
        nc.vector.reciprocal_approx_fast(out=rhoT[:], in_=stdT[:])
        rho10 = smalls.tile([P, NBLK], FP, tag="rho10")
        nc.vector.tensor_scalar(out=rho10[:], in0=rhoT[:], scalar1=SMOOTH,
                                scalar2=None, op0=Alu.mult)
        mu01 = smalls.tile([P, NBLK], FP, tag="mu01")
        nc.vector.tensor_scalar(out=mu01[:], in0=muT[:], scalar1=1.0 / SMOOTH,
                                scalar2=None, op0=Alu.mult)

        # ---------- FiLM params ----------
        bg1T = smalls.tile([P, NBLK], FP, tag="bg1T")
        nc.sync.dma_start(out=bg1T[:], in_=bg1T_d[:, :])
        bbT = smalls.tile([P, NBLK], FP, tag="bbT")
        nc.sync.dma_start(out=bbT[:], in_=bbT_d[:, :])

        g_all = smalls.tile([P, NBLK, CPC], FP, tag="g_all")
        bta_all = smalls.tile([P, NBLK, CPC], FP, tag="bta_all")
        with tc.tile_pool(name="gb_ps", bufs=4, space="PSUM") as gb_ps_pool:
            for which, wd, dst in (("g", wgT_d, g_all), ("b", wbT_d, bta_all)):
                for half in range(2):
                    ps = gb_ps_pool.tile([CPC, 512], FP, tag="gcd",
                                         name="gcd_ps")
                    for kb in range(NBLK):
                        if half == 0:
                            w = w0[(which, kb)]
                        else:
                            w = w_pool.tile([P, D // 2], FP, tag="w", name="w")
                            nc.sync.dma_start(
                                out=w[:], in_=wd[P * kb:P * (kb + 1),
                                                 512 * half:512 * (half + 1)])
                        nc.tensor.matmul(ps[:], capT[kb][:], w[:],
                                         start=(kb == 0),
                                         stop=(kb == NBLK - 1),
                                         skip_group_check=True)
                    gsb = gcd_pool.tile([CPC, 512], FP, tag="gsb", name="gsb")
                    nc.scalar.copy(gsb[:], ps[:])
                    for j in range(4):
                        db = half * 4 + j
                        pst = tp_psum.tile([P, P], FP, tag="tp")
                        nc.tensor.transpose(pst[:, 0:CPC],
                                            gsb[:, P * j:P * (j + 1)],
                                            ident[:CPC, :CPC])
                        nc.vector.tensor_copy(out=dst[:, db, :],
                                              in_=pst[:, 0:CPC])
        w0_cm.__exit__(None, None, None)

        def bcast_col(t2d):
            return t2d[:].rearrange("p b -> p b ()").broadcast_to(
                [P, NBLK, CPC])

        # a10 = (g + bg1)*rho10 (the raw temperature), aT = a10/10
        a10 = smalls.tile([P, NBLK, CPC], FP, tag="a10")
        t_a = smalls.tile([P, NBLK, CPC], FP, tag="t_a")
        nc.vector.tensor_tensor(out=t_a[:], in0=g_all[:], in1=bcast_col(bg1T),
                                op=Alu.add)
        nc.vector.tensor_tensor(out=a10[:], in0=t_a[:], in1=bcast_col(rho10),
                                op=Alu.mult)
        aT = smalls.tile([P, NBLK, CPC], FP, tag="aT")
        nc.vector.tensor_scalar(out=aT[:], in0=a10[:], scalar1=1.0 / SMOOTH,
                                scalar2=None, op0=Alu.mult)
        # capped temperature s' (used only as the exp scale)
        sc_all = smalls.tile([P, NBLK, CPC], FP, tag="sc_all")
        for blk in range(NBLK):
            nc.vector.tensor_scalar(out=sc_all[:, blk, :],
                                    in0=a10[:, blk, :],
                                    scalar1=thrS[:, blk:blk + 1],
                                    scalar2=nthrS[:, blk:blk + 1],
                                    op0=Alu.min, op1=Alu.max)
        # negb2 = a10*mu01 - (bta + bb)   (u = wa - negb2)
        negb2 = smalls.tile([P, NBLK, CPC], FP, tag="negb2")
        t_b = smalls.tile([P, NBLK, CPC], FP, tag="t_b")
        nc.vector.tensor_tensor(out=t_b[:], in0=bta_all[:],
                                in1=bcast_col(bbT), op=Alu.add)
        t_c2 = smalls.tile([P, NBLK, CPC], FP, tag="t_c2")
        nc.vector.tensor_tensor(out=t_c2[:], in0=a10[:], in1=bcast_col(mu01),
                                op=Alu.mult)
        nc.vector.tensor_tensor(out=negb2[:], in0=t_c2[:], in1=t_b[:],
                                op=Alu.subtract)
        # pre-expanded (over images) copies for the contiguous u-assembly
        aT_exp = smalls.tile([P, NBLK, CPC, B], BF, tag="aT_exp")
        nc.vector.tensor_copy(
            out=aT_exp[:],
            in_=aT[:].rearrange("p b c -> p b c ()").broadcast_to(
                [P, NBLK, CPC, B]))
        nb2_exp = smalls.tile([P, NBLK, CPC, B], BF, tag="nb2_exp")
        nc.vector.tensor_copy(
            out=nb2_exp[:],
            in_=negb2[:].rearrange("p b c -> p b c ()").broadcast_to(
                [P, NBLK, CPC, B]))

        # ---------- main loop ----------
        big_pool = ctx.enter_context(tc.tile_pool(name="big", bufs=3))
        sw_pool = ctx.enter_context(tc.tile_pool(name="sw", bufs=2))
        dots_sb = smalls.tile([CPC, B], FP, tag="dots_sb")
        usq_sb = smalls.tile([CPC, B], FP, tag="usq_sb")
        with tc.tile_pool(name="du_ps", bufs=3, space="PSUM") as du_ps_pool:
            for c in range(CPC):
                F9e = sw_pool.tile([P, NBLK, 9, B], BF, tag="F9e")
                F9p = sw_pool.tile([P, NBLK, 9, B], BF, tag="F9p")
                ps_du = du_ps_pool.tile([2, 2 * B], FP, tag="du")
                for blk in range(NBLK):
                    e = big_pool.tile([P, R, B], BF, tag="e", bufs=3)
                    nc.scalar.activation(e[:], xall[blk][:], Act.Exp,
                                         scale=sc_all[:, blk, c:c + 1])
                    p = big_pool.tile([P, R, B], BF, tag="p", bufs=3)
                    nc.vector.tensor_tensor(out=p[:], in0=e[:],
                                            in1=xall[blk][:], op=Alu.mult)
                    f18e = big_pool.tile([P, 18, B], BF, tag="f18e", bufs=3)
                    nc.vector.tensor_tensor(out=f18e[:], in0=e[:, 0:18, :],
                                            in1=e[:, 18:36, :], op=Alu.add)
                    nc.vector.tensor_tensor(out=F9e[:, blk],
                                            in0=f18e[:, 0:9, :],
                                            in1=f18e[:, 9:18, :], op=Alu.add)
                    geng = nc.gpsimd if PFOLD_GPSIMD else nc.vector
                    f18p = big_pool.tile([P, 18, B], BF, tag="f18p", bufs=3)
                    geng.tensor_tensor(out=f18p[:], in0=p[:, 0:18, :],
                                       in1=p[:, 18:36, :], op=Alu.add)
                    geng.tensor_tensor(out=F9p[:, blk],
                                       in0=f18p[:, 0:9, :],
                                       in1=f18p[:, 9:18, :], op=Alu.add)
                # batched fold tails: 9 -> (4+4+1) -> 2 -> 1, all blocks at
                # once, folding in place inside the F9 accumulators
                Sall = sw_pool.tile([P, NBLK, B], FP, tag="Sall")
                Wall = sw_pool.tile([P, NBLK, B], FP, tag="Wall")
                for F9, dst in ((F9e, Sall), (F9p, Wall)):
                    nc.vector.tensor_tensor(out=F9[:, :, 0:4, :],
                                            in0=F9[:, :, 0:4, :],
                                            in1=F9[:, :, 4:8, :], op=Alu.add)
                    nc.vector.tensor_tensor(out=F9[:, :, 0:2, :],
                                            in0=F9[:, :, 0:2, :],
                                            in1=F9[:, :, 2:4, :], op=Alu.add)
                    nc.vector.tensor_tensor(out=F9[:, :, 0, :],
                                            in0=F9[:, :, 0, :],
                                            in1=F9[:, :, 1, :], op=Alu.add)
                    nc.vector.tensor_tensor(out=dst[:], in0=F9[:, :, 0, :],
                                            in1=F9[:, :, 8, :], op=Alu.add)
                rs = sw_pool.tile([P, NBLK, B], FP, tag="rs")
                nc.vector.reciprocal_approx_fast(
                    out=rs[:].rearrange("p b i -> p (b i)"),
                    in_=Sall[:].rearrange("p b i -> p (b i)"))
                t1 = sw_pool.tile([P, NBLK, B], BF, tag="t1")
                nc.vector.tensor_tensor(out=t1[:], in0=Wall[:], in1=rs[:],
                                        op=Alu.mult)
                uu_big = sw_pool.tile([P, 2, NBLK, B], BF, tag="uu")
                t2 = sw_pool.tile([P, NBLK, B], BF, tag="t2")
                nc.vector.tensor_tensor(out=t2[:], in0=t1[:],
                                        in1=aT_exp[:, :, c, :], op=Alu.mult)
                nc.vector.tensor_tensor(out=uu_big[:, 0], in0=t2[:],
                                        in1=nb2_exp[:, :, c, :],
                                        op=Alu.subtract)
                nc.vector.tensor_tensor(out=uu_big[:, 1], in0=uu_big[:, 0],
                                        in1=uu_big[:, 0], op=Alu.mult)
                for blk in range(NBLK):
                    nc.tensor.matmul(ps_du[:], capT2[blk][:, c, :],
                                     uu_big[:, :, blk, :],
                                     start=(blk == 0), stop=(blk == NBLK - 1),
                                     skip_group_check=True)
                du_c = sw_pool.tile([2, 2 * B], FP, tag="du_c", bufs=2)
                nc.scalar.copy(du_c[:], ps_du[:])
                nc.sync.dma_start(out=dots_sb[c:c + 1, :],
                                  in_=du_c[0:1, 0:B])
                nc.sync.dma_start(out=usq_sb[c:c + 1, :],
                                  in_=du_c[1:2, B:2 * B])

        # ---------- tail ----------
        sq = smalls.tile([CPC, B], FP, tag="sqf")
        nc.scalar.activation(sq[:], usq_sb[:], Act.Sqrt)
        ru = smalls.tile([CPC, B], FP, tag="ruf")
        nc.vector.reciprocal_approx_fast(out=ru[:], in_=sq[:])
        t3 = smalls.tile([CPC, B], FP, tag="t3f")
        nc.vector.tensor_tensor(out=t3[:], in0=dots_sb[:], in1=ru[:],
                                op=Alu.mult)
        out_sb = smalls.tile([CPC, B], FP, tag="out_sb")
        nc.vector.tensor_scalar(out=out_sb[:], in0=t3[:], scalar1=rn[:, 0:1],
                                scalar2=None, op0=Alu.mult)
        nc.sync.dma_start(out=out_d[:, :], in_=out_sb[:])

    nc.compile()
    return nc


def _get_nc():
    if "nc" not in _CACHE:
        _CACHE["nc"] = _build_nc()
    return _CACHE["nc"]


def kernel(img_embed, cap_embed, lens, W_gamma, b_gamma, W_beta, b_beta,
           _want_trace=False):
    from concourse.bass_utils import run_bass_kernel_spmd

    nc = _get_nc()

    img_embed = np.asarray(img_embed, np.float32)
    cap_embed = np.asarray(cap_embed, np.float32)
    lens_np = np.asarray(lens)
    W_gamma = np.asarray(W_gamma, np.float32)
    W_beta = np.asarray(W_beta, np.float32)
    b_gamma = np.asarray(b_gamma, np.float32)
    b_beta = np.asarray(b_beta, np.float32)

    import ml_dtypes
    # rows (r, i)-major so the on-chip tiles are [P, R, B]
    img_ri = img_embed.transpose(1, 0, 2).reshape(NIR, D)
    img_bf = np.ascontiguousarray(img_ri.astype(ml_dtypes.bfloat16))
    img_t2 = np.ascontiguousarray(img_bf.T)
    wgT = np.ascontiguousarray(W_gamma.T)
    wbT = np.ascontiguousarray(W_beta.T)
    bg1T = np.ascontiguousarray((1.0 + b_gamma).reshape(NBLK, P).T)
    bbT = np.ascontiguousarray(b_beta.reshape(NBLK, P).T)

    lens_f = lens_np.astype(np.float64)
    mask = (np.arange(T)[None, :] < lens_np[:, None]).astype(np.float64)
    mask = (mask / lens_f[:, None]).astype(np.float32)  # (B, T)

    in_maps = []
    for k in range(NCORES):
        sl = slice(k * CPC, (k + 1) * CPC)
        in_maps.append({
            "imgbf": img_bf,
            "imgT2": img_t2,
            "cap": np.ascontiguousarray(cap_embed[sl]),
            "maskT": np.ascontiguousarray(mask[sl].T),
            "wgT": wgT,
            "wbT": wbT,
            "bg1T": bg1T,
            "bbT": bbT,
        })

    kw = {}
    if _want_trace:
        import os as _os2, shutil as _sh
        _sh.rmtree("/tmp/ktrace", ignore_errors=True)
        _os2.makedirs("/tmp/ktrace", exist_ok=True)
        kw = {"tmpdir": "/tmp/ktrace"}
    res = run_bass_kernel_spmd(nc, in_maps, core_ids=list(range(NCORES)),
                               trace=_want_trace, **kw)
    outs = [np.asarray(r["out"]) for r in res.results]
    sims = np.concatenate([o.T for o in outs], axis=1).astype(np.float32)
    if _want_trace:
        return sims, res
    return sims


# revision 36
# speedup vs baseline: 1.2936x; 1.0700x over previous
"""AdaptiveEmbedding T2I sims kernel for 8 TRN2 NeuronCores.

Sharding: caption batch 48 -> 6 per core; every core holds the full image
tensor and emits a [6, 48] slice of sims^T; host concatenates + transposes.

Math (see comments): BN folds into the FiLM affine; the softmax max-shift
cancels in the weighted mean; beta re-enters linearly. exp overflow is
prevented by capping the per-(c,d) softmax temperature s = 10*a at
+-CLAMP/X*_d where X*_d = max|x| over (i,r) for channel d — rows beyond
the cap keep their region ordering at temperature CLAMP/X* (strictly less
distortion than pointwise clamping of s*x, which flattens all clamped
regions to equal weights).

Layouts: image tiles are r-major [P, R, B] (host permutes rows to
(r, i)-major) so every fold of the region dimension is a contiguous
half-add that hits the DVE 2x bf16 mode.

Main loop per (caption c, channel block b):
  ACT   : e = exp(s'*x)            (no clamp, no intermediate)
  DVE   : p = e*x (2x), e-fold 36->18->9 (2x), strided reduce9 -> S,
          strided reduce9 of p-folds -> W
  GpSimd: p-fold 36->18->9         (keeps DVE below the ACT+fold wall)
  PE    : [capT_c | ones]^T @ [u ; u*u] accumulated over blocks -> psum
Per caption: batched fast-reciprocal of S, u assembly as 8 dual-scalar
TS ops, one uu multiply.
"""

import numpy as np
from contextlib import ExitStack

B, T, D, R = 48, 50, 1024, 36
NCORES = 8
CPC = B // NCORES          # captions per core
SMOOTH = 10.0
CLAMP = 80.0
BN_EPS = 1e-5
L2_EPS = 1e-8
P = 128
NBLK = D // P              # 8 channel blocks
NIR = B * R                # 1728 rows
NCH = (NIR + P - 1) // P   # 14 native-layout chunks

_CACHE = {}

# knobs
PFOLD_GPSIMD = True        # p-fold chain on GpSimd (else DVE)
STRIDED_RED9 = True        # strided-view reduce of f9 (else fold to end)


def _build_nc():
    import concourse.bass as bass
    import concourse.tile as tile
    from concourse import bacc, mybir
    from concourse.masks import make_identity

    FP = mybir.dt.float32
    BF = mybir.dt.bfloat16
    Alu = mybir.AluOpType
    Act = mybir.ActivationFunctionType
    Ax = mybir.AxisListType

    nc = bacc.Bacc("TRN2", target_bir_lowering=False, debug=False,
                   num_devices=NCORES)

    # imgbf rows are (r, i)-major: row r*B+i = img[i, r, :]
    imgbf = nc.dram_tensor("imgbf", (NIR, D), BF, kind="ExternalInput").ap()
    # imgT2 is the d-major copy: row d = img[:, :, d] in (r, i) order
    imgT2 = nc.dram_tensor("imgT2", (D, NIR), BF, kind="ExternalInput").ap()
    cap = nc.dram_tensor("cap", (CPC, T, D), FP, kind="ExternalInput").ap()
    maskT_d = nc.dram_tensor("maskT", (T, CPC), FP, kind="ExternalInput").ap()
    wgT_d = nc.dram_tensor("wgT", (D, D), FP, kind="ExternalInput").ap()
    wbT_d = nc.dram_tensor("wbT", (D, D), FP, kind="ExternalInput").ap()
    bg1T_d = nc.dram_tensor("bg1T", (P, NBLK), FP, kind="ExternalInput").ap()
    bbT_d = nc.dram_tensor("bbT", (P, NBLK), FP, kind="ExternalInput").ap()
    out_d = nc.dram_tensor("out", (CPC, B), FP, kind="ExternalOutput").ap()

    with tile.TileContext(nc) as tc, ExitStack() as ctx:
        consts = ctx.enter_context(tc.tile_pool(name="consts", bufs=1))
        ident = consts.tile([P, P], FP, tag="ident")
        make_identity(nc, ident[:])
        ones1b = consts.tile([P, 1], BF, tag="ones1b")
        nc.vector.memset(ones1b[:], 1.0)

        smalls = ctx.enter_context(tc.tile_pool(name="smalls", bufs=1))
        cap_pool = ctx.enter_context(tc.tile_pool(name="cap", bufs=3))
        tp_psum = ctx.enter_context(tc.tile_pool(name="tp_ps", bufs=2,
                                                 space="PSUM"))
        sq_pool = ctx.enter_context(tc.tile_pool(name="sq", bufs=2))
        w_pool = ctx.enter_context(tc.tile_pool(name="w", bufs=3))
        gcd_pool = ctx.enter_context(tc.tile_pool(name="gcd", bufs=2))
        xall_pool = ctx.enter_context(tc.tile_pool(name="xall", bufs=1))

        # image tiles first — they gate X*/stats and the main loop;
        # both layouts come pre-transposed from the host: plain row DMAs only
        xall = [xall_pool.tile([P, R, B], BF, tag=f"xall{b}", name=f"xall{b}")
                for b in range(NBLK)]
        for blk in range(NBLK):
            nc.sync.dma_start(
                out=xall[blk][:].rearrange("p r i -> p (r i)"),
                in_=imgT2[P * blk:P * (blk + 1), :])

        # ---------- caption branch ----------
        maskT = smalls.tile([T, CPC], FP, tag="maskT")
        nc.sync.dma_start(out=maskT[:], in_=maskT_d[:, :])
        cap_sb = smalls.tile([CPC, D], FP, tag="cap_sb")
        with tc.tile_pool(name="cap_ps", bufs=2, space="PSUM") as cap_ps_pool:
            for c in range(CPC):
                ct = cap_pool.tile([T, D], FP, tag="cap")
                nc.sync.dma_start(out=ct[:], in_=cap[c, :, :])
                pp = cap_ps_pool.tile([1, D], FP, tag="pp", name="pp")
                for j in range(2):
                    nc.tensor.matmul(pp[:, 512 * j:512 * (j + 1)],
                                     maskT[:, c:c + 1],
                                     ct[:, 512 * j:512 * (j + 1)],
                                     start=True, stop=True,
                                     skip_group_check=True)
                prow = cap_pool.tile([1, D], FP, tag="prow", name="prow",
                                     bufs=2)
                nc.scalar.copy(prow[:], pp[:])
                nc.sync.dma_start(out=cap_sb[c:c + 1, :], in_=prow[:])

        # weight half-0 loads issued early (before the transpose-DMA flood);
        # pool closed manually right after the FiLM matmuls free the space
        w0_cm = tc.tile_pool(name="w0", bufs=1)
        w0_pool = w0_cm.__enter__()
        w0 = {}
        for which, wd in (("g", wgT_d), ("b", wbT_d)):
            for kb in range(NBLK):
                t = w0_pool.tile([P, D // 2], FP, tag=f"w0{which}{kb}",
                                 name=f"w0{which}{kb}")
                nc.sync.dma_start(out=t[:], in_=wd[P * kb:P * (kb + 1), 0:512])
                w0[(which, kb)] = t

        capT = [smalls.tile([P, CPC], FP, tag=f"capT{b}", name=f"capT{b}")
                for b in range(NBLK)]
        capT2 = [smalls.tile([P, CPC, 2], BF, tag=f"capT2{b}",
                             name=f"capT2{b}") for b in range(NBLK)]
        for blk in range(NBLK):
            nc.vector.memset(capT2[blk][:], 1.0)
            pst = tp_psum.tile([P, P], FP, tag="tp")
            nc.tensor.transpose(pst[:, 0:CPC], cap_sb[:, P * blk:P * (blk + 1)],
                                ident[:CPC, :CPC])
            nc.vector.tensor_copy(out=capT[blk][:], in_=pst[:, 0:CPC])
            nc.vector.tensor_copy(out=capT2[blk][:, :, 0], in_=pst[:, 0:CPC])

        scr_c = smalls.tile([CPC, D], FP, tag="scr_c")
        nc.vector.tensor_tensor(out=scr_c[:], in0=cap_sb[:], in1=cap_sb[:],
                                op=Alu.mult)
        n2 = smalls.tile([CPC, 1], FP, tag="n2")
        nc.vector.tensor_reduce(out=n2[:], in_=scr_c[:], axis=Ax.X, op=Alu.add)
        nrm = smalls.tile([CPC, 1], FP, tag="nrm")
        nc.scalar.activation(nrm[:], n2[:], Act.Sqrt)
        nrm_e = smalls.tile([CPC, 1], FP, tag="nrm_e")
        nc.vector.tensor_scalar(out=nrm_e[:], in0=nrm[:], scalar1=L2_EPS,
                                scalar2=None, op0=Alu.add)
        rn = smalls.tile([CPC, 1], FP, tag="rn")
        nc.vector.reciprocal(rn[:], nrm_e[:])

        # ---------- image loads ----------
        sums_sb = smalls.tile([1, D], FP, tag="sums_sb")
        muT = smalls.tile([P, NBLK], FP, tag="muT")
        with tc.tile_pool(name="xt", bufs=1) as xt_pool, \
             tc.tile_pool(name="xs_ps", bufs=1, space="PSUM") as xs_ps_pool:
            xt = [xt_pool.tile([P, D], BF, tag=f"xt{k}", name=f"xt{k}")
                  for k in range(NCH)]
            for k in range(NCH):
                rows = min(P, NIR - P * k)
                nc.sync.dma_start(out=xt[k][0:rows, :],
                                  in_=imgbf[P * k:P * k + rows, :])
            ps = [xs_ps_pool.tile([1, 512], FP, tag=f"xs{h}", name=f"xs{h}")
                  for h in range(2)]
            for h in range(2):
                for k in range(NCH):
                    rows = min(P, NIR - P * k)
                    nc.tensor.matmul(ps[h][:], ones1b[0:rows, :],
                                     xt[k][0:rows, 512 * h:512 * (h + 1)],
                                     start=(k == 0), stop=(k == NCH - 1),
                                     skip_group_check=True)
            for h in range(2):
                nc.scalar.copy(sums_sb[:, 512 * h:512 * (h + 1)], ps[h][:])
        inv_n = 1.0 / float(NIR)
        muT_raw = smalls.tile([P, NBLK], FP, tag="muT_raw")
        for b2 in range(NBLK):
            pst = tp_psum.tile([P, P], FP, tag="tp")
            nc.tensor.transpose(pst[:, 0:1],
                                sums_sb[:, P * b2:P * (b2 + 1)],
                                ident[:1, :1])
            nc.vector.tensor_copy(out=muT_raw[:, b2:b2 + 1], in_=pst[:, 0:1])
        nc.vector.tensor_scalar(out=muT[:], in0=muT_raw[:], scalar1=inv_n,
                                scalar2=None, op0=Alu.mult)

        # x^2 sums (DVE contiguous folds) and X* = max|x| (GpSimd folds)
        m2T = smalls.tile([P, NBLK], FP, tag="m2T")
        xstar = smalls.tile([P, NBLK], FP, tag="xstar")
        for blk in range(NBLK):
            xf = xall[blk][:].rearrange("p r i -> p (r i)")
            x2 = sq_pool.tile([P, NIR], BF, tag="x2")
            nc.scalar.square(x2[:], xf)
            f1 = sq_pool.tile([P, NIR // 2], BF, tag="f1")
            nc.vector.tensor_tensor(out=f1[:], in0=x2[:, 0:NIR // 2],
                                    in1=x2[:, NIR // 2:NIR], op=Alu.add)
            f2 = sq_pool.tile([P, NIR // 4], BF, tag="f2")
            nc.vector.tensor_tensor(out=f2[:], in0=f1[:, 0:NIR // 4],
                                    in1=f1[:, NIR // 4:NIR // 2], op=Alu.add)
            sq_s = sq_pool.tile([P, 1], FP, tag="sq_s", bufs=2)
            nc.vector.tensor_reduce(out=sq_s[:], in_=f2[:], axis=Ax.X,
                                    op=Alu.add)
            nc.vector.tensor_scalar(out=m2T[:, blk:blk + 1], in0=sq_s[:],
                                    scalar1=inv_n, scalar2=None, op0=Alu.mult)
            # X* = max|x| per channel, one absolute-value max-reduce
            nc.vector.tensor_reduce(out=xstar[:, blk:blk + 1], in_=xf,
                                    axis=Ax.X, op=Alu.max,
                                    apply_absolute_value=True)

        # per-channel temperature cap thrS = CLAMP / X*
        rxs = smalls.tile([P, NBLK], FP, tag="rxs")
        nc.vector.reciprocal_approx_fast(out=rxs[:], in_=xstar[:])
        thrS = smalls.tile([P, NBLK], FP, tag="thrS")
        nc.vector.tensor_scalar(out=thrS[:], in0=rxs[:], scalar1=CLAMP,
                                scalar2=None, op0=Alu.mult)
        nthrS = smalls.tile([P, NBLK], FP, tag="nthrS")
        nc.vector.tensor_scalar(out=nthrS[:], in0=thrS[:], scalar1=-1.0,
                                scalar2=None, op0=Alu.mult)

        # BN: rho10 = 10/sqrt(var+eps), mu01 = mu/10
        musqT = smalls.tile([P, NBLK], FP, tag="musqT")
        nc.scalar.square(musqT[:], muT[:])
        varT = smalls.tile([P, NBLK], FP, tag="varT")
        nc.vector.tensor_tensor(out=varT[:], in0=m2T[:], in1=musqT[:],
                                op=Alu.subtract)
        epsT = smalls.tile([P, 1], FP, tag="epsT")
        nc.vector.memset(epsT[:], BN_EPS)
        stdT = smalls.tile([P, NBLK], FP, tag="stdT")
        nc.scalar.activation(stdT[:], varT[:], Act.Sqrt, bias=epsT[:])
        rhoT = smalls.tile([P, NBLK], FP, tag="rhoT")
        nc.vector.reciprocal_approx_fast(out=rhoT[:], in_=stdT[:])
        rho10 = smalls.tile([P, NBLK], FP, tag="rho10")
        nc.vector.tensor_scalar(out=rho10[:], in0=rhoT[:], scalar1=SMOOTH,
                                scalar2=None, op0=Alu.mult)
        mu01 = smalls.tile([P, NBLK], FP, tag="mu01")
        nc.vector.tensor_scalar(out=mu01[:], in0=muT[:], scalar1=1.0 / SMOOTH,
                                scalar2=None, op0=Alu.mult)

        # ---------- FiLM params ----------
        bg1T = smalls.tile([P, NBLK], FP, tag="bg1T")
        nc.sync.dma_start(out=bg1T[:], in_=bg1T_d[:, :])
        bbT = smalls.tile([P, NBLK], FP, tag="bbT")
        nc.sync.dma_start(out=bbT[:], in_=bbT_d[:, :])

        g_all = smalls.tile([P, NBLK, CPC], FP, tag="g_all")
        bta_all = smalls.tile([P, NBLK, CPC], FP, tag="bta_all")
        with tc.tile_pool(name="gb_ps", bufs=4, space="PSUM") as gb_ps_pool:
            for which, wd, dst in (("g", wgT_d, g_all), ("b", wbT_d, bta_all)):
                for half in range(2):
                    ps = gb_ps_pool.tile([CPC, 512], FP, tag="gcd",
                                         name="gcd_ps")
                    for kb in range(NBLK):
                        if half == 0:
                            w = w0[(which, kb)]
                        else:
                            w = w_pool.tile([P, D // 2], FP, tag="w", name="w")
                            nc.sync.dma_start(
                                out=w[:], in_=wd[P * kb:P * (kb + 1),
                                                 512 * half:512 * (half + 1)])
                        nc.tensor.matmul(ps[:], capT[kb][:], w[:],
                                         start=(kb == 0),
                                         stop=(kb == NBLK - 1),
                                         skip_group_check=True)
                    gsb = gcd_pool.tile([CPC, 512], FP, tag="gsb", name="gsb")
                    nc.scalar.copy(gsb[:], ps[:])
                    for j in range(4):
                        db = half * 4 + j
                        pst = tp_psum.tile([P, P], FP, tag="tp")
                        nc.tensor.transpose(pst[:, 0:CPC],
                                            gsb[:, P * j:P * (j + 1)],
                                            ident[:CPC, :CPC])
                        nc.vector.tensor_copy(out=dst[:, db, :],
                                              in_=pst[:, 0:CPC])
        w0_cm.__exit__(None, None, None)

        def bcast_col(t2d):
            return t2d[:].rearrange("p b -> p b ()").broadcast_to(
                [P, NBLK, CPC])

        # a10 = (g + bg1)*rho10 (the raw temperature), aT = a10/10
        a10 = smalls.tile([P, NBLK, CPC], FP, tag="a10")
        t_a = smalls.tile([P, NBLK, CPC], FP, tag="t_a")
        nc.vector.tensor_tensor(out=t_a[:], in0=g_all[:], in1=bcast_col(bg1T),
                                op=Alu.add)
        nc.vector.tensor_tensor(out=a10[:], in0=t_a[:], in1=bcast_col(rho10),
                                op=Alu.mult)
        aT = smalls.tile([P, NBLK, CPC], FP, tag="aT")
        nc.vector.tensor_scalar(out=aT[:], in0=a10[:], scalar1=1.0 / SMOOTH,
                                scalar2=None, op0=Alu.mult)
        # capped temperature s' (used only as the exp scale)
        sc_all = smalls.tile([P, NBLK, CPC], FP, tag="sc_all")
        for blk in range(NBLK):
            nc.vector.tensor_scalar(out=sc_all[:, blk, :],
                                    in0=a10[:, blk, :],
                                    scalar1=thrS[:, blk:blk + 1],
                                    scalar2=nthrS[:, blk:blk + 1],
                                    op0=Alu.min, op1=Alu.max)
        # negb2 = a10*mu01 - (bta + bb)   (u = wa - negb2)
        negb2 = smalls.tile([P, NBLK, CPC], FP, tag="negb2")
        t_b = smalls.tile([P, NBLK, CPC], FP, tag="t_b")
        nc.vector.tensor_tensor(out=t_b[:], in0=bta_all[:],
                                in1=bcast_col(bbT), op=Alu.add)
        t_c2 = smalls.tile([P, NBLK, CPC], FP, tag="t_c2")
        nc.vector.tensor_tensor(out=t_c2[:], in0=a10[:], in1=bcast_col(mu01),
                                op=Alu.mult)
        nc.vector.tensor_tensor(out=negb2[:], in0=t_c2[:], in1=t_b[:],
                                op=Alu.subtract)
        # pre-expanded (over images) copies for the contiguous u-assembly
        aT_exp = smalls.tile([P, NBLK, CPC, B], BF, tag="aT_exp")
        nc.vector.tensor_copy(
            out=aT_exp[:],
            in_=aT[:].rearrange("p b c -> p b c ()").broadcast_to(
                [P, NBLK, CPC, B]))
        nb2_exp = smalls.tile([P, NBLK, CPC, B], BF, tag="nb2_exp")
        nc.vector.tensor_copy(
            out=nb2_exp[:],
            in_=negb2[:].rearrange("p b c -> p b c ()").broadcast_to(
                [P, NBLK, CPC, B]))

        # ---------- main loop ----------
        big_pool = ctx.enter_context(tc.tile_pool(name="big", bufs=3))
        sw_pool = ctx.enter_context(tc.tile_pool(name="sw", bufs=2))
        dots_sb = smalls.tile([CPC, B], FP, tag="dots_sb")
        usq_sb = smalls.tile([CPC, B], FP, tag="usq_sb")
        with tc.tile_pool(name="du_ps", bufs=3, space="PSUM") as du_ps_pool:
            for c in range(CPC):
                F9e = sw_pool.tile([P, NBLK, 9, B], BF, tag="F9e")
                F9p = sw_pool.tile([P, NBLK, 9, B], BF, tag="F9p")
                ps_du = du_ps_pool.tile([2, 2 * B], FP, tag="du")
                for blk in range(NBLK):
                    e = big_pool.tile([P, R, B], BF, tag="e", bufs=3)
                    nc.scalar.activation(e[:], xall[blk][:], Act.Exp,
                                         scale=sc_all[:, blk, c:c + 1])
                    p = big_pool.tile([P, R, B], BF, tag="p", bufs=3)
                    nc.vector.tensor_tensor(out=p[:], in0=e[:],
                                            in1=xall[blk][:], op=Alu.mult)
                    geng = nc.gpsimd if PFOLD_GPSIMD else nc.vector
                    f18e = big_pool.tile([P, 18, B], BF, tag="f18e", bufs=3)
                    nc.vector.tensor_tensor(out=f18e[:], in0=e[:, 0:18, :],
                                            in1=e[:, 18:36, :], op=Alu.add)
                    geng.tensor_tensor(out=F9e[:, blk],
                                       in0=f18e[:, 0:9, :],
                                       in1=f18e[:, 9:18, :], op=Alu.add)
                    f18p = big_pool.tile([P, 18, B], BF, tag="f18p", bufs=3)
                    nc.vector.tensor_tensor(out=f18p[:], in0=p[:, 0:18, :],
                                            in1=p[:, 18:36, :], op=Alu.add)
                    geng.tensor_tensor(out=F9p[:, blk],
                                       in0=f18p[:, 0:9, :],
                                       in1=f18p[:, 9:18, :], op=Alu.add)
                # batched fold tails: 9 -> (4+4+1) -> 2 -> 1, all blocks at
                # once, folding in place inside the F9 accumulators
                Sall = sw_pool.tile([P, NBLK, B], FP, tag="Sall")
                Wall = sw_pool.tile([P, NBLK, B], FP, tag="Wall")
                for F9, dst in ((F9e, Sall), (F9p, Wall)):
                    nc.vector.tensor_tensor(out=F9[:, :, 0:4, :],
                                            in0=F9[:, :, 0:4, :],
                                            in1=F9[:, :, 4:8, :], op=Alu.add)
                    nc.vector.tensor_tensor(out=F9[:, :, 0:2, :],
                                            in0=F9[:, :, 0:2, :],
                                            in1=F9[:, :, 2:4, :], op=Alu.add)
                    nc.vector.tensor_tensor(out=F9[:, :, 0, :],
                                            in0=F9[:, :, 0, :],
                                            in1=F9[:, :, 1, :], op=Alu.add)
                    nc.vector.tensor_tensor(out=dst[:], in0=F9[:, :, 0, :],
                                            in1=F9[:, :, 8, :], op=Alu.add)
                rs = sw_pool.tile([P, NBLK, B], FP, tag="rs")
                nc.vector.reciprocal_approx_fast(
                    out=rs[:].rearrange("p b i -> p (b i)"),
                    in_=Sall[:].rearrange("p b i -> p (b i)"))
                t1 = sw_pool.tile([P, NBLK, B], BF, tag="t1")
                nc.vector.tensor_tensor(out=t1[:], in0=Wall[:], in1=rs[:],
                                        op=Alu.mult)
                uu_big = sw_pool.tile([P, 2, NBLK, B], BF, tag="uu")
                t2 = sw_pool.tile([P, NBLK, B], BF, tag="t2")
                nc.vector.tensor_tensor(out=t2[:], in0=t1[:],
                                        in1=aT_exp[:, :, c, :], op=Alu.mult)
                nc.vector.tensor_tensor(out=uu_big[:, 0], in0=t2[:],
                                        in1=nb2_exp[:, :, c, :],
                                        op=Alu.subtract)
                nc.vector.tensor_tensor(out=uu_big[:, 1], in0=uu_big[:, 0],
                                        in1=uu_big[:, 0], op=Alu.mult)
                for blk in range(NBLK):
                    nc.tensor.matmul(ps_du[:], capT2[blk][:, c, :],
                                     uu_big[:, :, blk, :],
                                     start=(blk == 0), stop=(blk == NBLK - 1),
                                     skip_group_check=True)
                du_c = sw_pool.tile([2, 2 * B], FP, tag="du_c", bufs=2)
                nc.scalar.copy(du_c[:], ps_du[:])
                nc.sync.dma_start(out=dots_sb[c:c + 1, :],
                                  in_=du_c[0:1, 0:B])
                nc.sync.dma_start(out=usq_sb[c:c + 1, :],
                                  in_=du_c[1:2, B:2 * B])

        # ---------- tail ----------
        sq = smalls.tile([CPC, B], FP, tag="sqf")
        nc.scalar.activation(sq[:], usq_sb[:], Act.Sqrt)
        ru = smalls.tile([CPC, B], FP, tag="ruf")
        nc.vector.reciprocal_approx_fast(out=ru[:], in_=sq[:])
        t3 = smalls.tile([CPC, B], FP, tag="t3f")
        nc.vector.tensor_tensor(out=t3[:], in0=dots_sb[:], in1=ru[:],
                                op=Alu.mult)
        out_sb = smalls.tile([CPC, B], FP, tag="out_sb")
        nc.vector.tensor_scalar(out=out_sb[:], in0=t3[:], scalar1=rn[:, 0:1],
                                scalar2=None, op0=Alu.mult)
        nc.sync.dma_start(out=out_d[:, :], in_=out_sb[:])

    nc.compile()
    return nc


def _get_nc():
    if "nc" not in _CACHE:
        _CACHE["nc"] = _build_nc()
    return _CACHE["nc"]


def kernel(img_embed, cap_embed, lens, W_gamma, b_gamma, W_beta, b_beta,
           _want_trace=False):
    from concourse.bass_utils import run_bass_kernel_spmd

    nc = _get_nc()

    img_embed = np.asarray(img_embed, np.float32)
    cap_embed = np.asarray(cap_embed, np.float32)
    lens_np = np.asarray(lens)
    W_gamma = np.asarray(W_gamma, np.float32)
    W_beta = np.asarray(W_beta, np.float32)
    b_gamma = np.asarray(b_gamma, np.float32)
    b_beta = np.asarray(b_beta, np.float32)

    import ml_dtypes
    # rows (r, i)-major so the on-chip tiles are [P, R, B]
    img_ri = img_embed.transpose(1, 0, 2).reshape(NIR, D)
    img_bf = np.ascontiguousarray(img_ri.astype(ml_dtypes.bfloat16))
    img_t2 = np.ascontiguousarray(img_bf.T)
    wgT = np.ascontiguousarray(W_gamma.T)
    wbT = np.ascontiguousarray(W_beta.T)
    bg1T = np.ascontiguousarray((1.0 + b_gamma).reshape(NBLK, P).T)
    bbT = np.ascontiguousarray(b_beta.reshape(NBLK, P).T)

    lens_f = lens_np.astype(np.float64)
    mask = (np.arange(T)[None, :] < lens_np[:, None]).astype(np.float64)
    mask = (mask / lens_f[:, None]).astype(np.float32)  # (B, T)

    in_maps = []
    for k in range(NCORES):
        sl = slice(k * CPC, (k + 1) * CPC)
        in_maps.append({
            "imgbf": img_bf,
            "imgT2": img_t2,
            "cap": np.ascontiguousarray(cap_embed[sl]),
            "maskT": np.ascontiguousarray(mask[sl].T),
            "wgT": wgT,
            "wbT": wbT,
            "bg1T": bg1T,
            "bbT": bbT,
        })

    kw = {}
    if _want_trace:
        import os as _os2, shutil as _sh
        _sh.rmtree("/tmp/ktrace", ignore_errors=True)
        _os2.makedirs("/tmp/ktrace", exist_ok=True)
        kw = {"tmpdir": "/tmp/ktrace"}
    res = run_bass_kernel_spmd(nc, in_maps, core_ids=list(range(NCORES)),
                               trace=_want_trace, **kw)
    outs = [np.asarray(r["out"]) for r in res.results]
    sims = np.concatenate([o.T for o in outs], axis=1).astype(np.float32)
    if _want_trace:
        return sims, res
    return sims


# revision 37
# speedup vs baseline: 1.3219x; 1.0219x over previous
"""AdaptiveEmbedding T2I sims kernel for 8 TRN2 NeuronCores.

Sharding: caption batch 48 -> 6 per core; every core holds the full image
tensor and emits a [6, 48] slice of sims^T; host concatenates + transposes.

Math (see comments): BN folds into the FiLM affine; the softmax max-shift
cancels in the weighted mean; beta re-enters linearly. exp overflow is
prevented by capping the per-(c,d) softmax temperature s = 10*a at
+-CLAMP/X*_d where X*_d = max|x| over (i,r) for channel d — rows beyond
the cap keep their region ordering at temperature CLAMP/X* (strictly less
distortion than pointwise clamping of s*x, which flattens all clamped
regions to equal weights).

Layouts: image tiles are r-major [P, R, B] (host permutes rows to
(r, i)-major) so every fold of the region dimension is a contiguous
half-add that hits the DVE 2x bf16 mode.

Main loop per (caption c, channel block b):
  ACT   : e = exp(s'*x)            (no clamp, no intermediate)
  DVE   : p = e*x (2x), e-fold 36->18->9 (2x), strided reduce9 -> S,
          strided reduce9 of p-folds -> W
  GpSimd: p-fold 36->18->9         (keeps DVE below the ACT+fold wall)
  PE    : [capT_c | ones]^T @ [u ; u*u] accumulated over blocks -> psum
Per caption: batched fast-reciprocal of S, u assembly as 8 dual-scalar
TS ops, one uu multiply.
"""

import numpy as np
from contextlib import ExitStack

B, T, D, R = 48, 50, 1024, 36
NCORES = 8
CPC = B // NCORES          # captions per core
SMOOTH = 10.0
CLAMP = 80.0
BN_EPS = 1e-5
L2_EPS = 1e-8
P = 128
NBLK = D // P              # 8 channel blocks
NIR = B * R                # 1728 rows
NCH = (NIR + P - 1) // P   # 14 native-layout chunks

_CACHE = {}

# knobs
PFOLD_GPSIMD = True        # p-fold chain on GpSimd (else DVE)
STRIDED_RED9 = True        # strided-view reduce of f9 (else fold to end)


def _build_nc():
    import concourse.bass as bass
    import concourse.tile as tile
    from concourse import bacc, mybir
    from concourse.masks import make_identity

    FP = mybir.dt.float32
    BF = mybir.dt.bfloat16
    Alu = mybir.AluOpType
    Act = mybir.ActivationFunctionType
    Ax = mybir.AxisListType

    nc = bacc.Bacc("TRN2", target_bir_lowering=False, debug=False,
                   num_devices=NCORES)

    # imgbf rows are (r, i)-major: row r*B+i = img[i, r, :]
    imgbf = nc.dram_tensor("imgbf", (NIR, D), BF, kind="ExternalInput").ap()
    # imgT2 is the d-major copy: row d = img[:, :, d] in (r, i) order
    imgT2 = nc.dram_tensor("imgT2", (D, NIR), BF, kind="ExternalInput").ap()
    cap = nc.dram_tensor("cap", (CPC, T, D), FP, kind="ExternalInput").ap()
    maskT_d = nc.dram_tensor("maskT", (T, CPC), FP, kind="ExternalInput").ap()
    wgT_d = nc.dram_tensor("wgT", (D, D), FP, kind="ExternalInput").ap()
    wbT_d = nc.dram_tensor("wbT", (D, D), FP, kind="ExternalInput").ap()
    bg1T_d = nc.dram_tensor("bg1T", (P, NBLK), FP, kind="ExternalInput").ap()
    bbT_d = nc.dram_tensor("bbT", (P, NBLK), FP, kind="ExternalInput").ap()
    out_d = nc.dram_tensor("out", (CPC, B), FP, kind="ExternalOutput").ap()

    with tile.TileContext(nc) as tc, ExitStack() as ctx:
        consts = ctx.enter_context(tc.tile_pool(name="consts", bufs=1))
        ident = consts.tile([P, P], FP, tag="ident")
        make_identity(nc, ident[:])
        ones1b = consts.tile([P, 1], BF, tag="ones1b")
        nc.vector.memset(ones1b[:], 1.0)

        smalls = ctx.enter_context(tc.tile_pool(name="smalls", bufs=1))
        cap_pool = ctx.enter_context(tc.tile_pool(name="cap", bufs=3))
        tp_psum = ctx.enter_context(tc.tile_pool(name="tp_ps", bufs=2,
                                                 space="PSUM"))
        sq_pool = ctx.enter_context(tc.tile_pool(name="sq", bufs=2))
        w_pool = ctx.enter_context(tc.tile_pool(name="w", bufs=3))
        gcd_pool = ctx.enter_context(tc.tile_pool(name="gcd", bufs=2))
        xall_pool = ctx.enter_context(tc.tile_pool(name="xall", bufs=1))

        # image tiles first — they gate X*/stats and the main loop;
        # both layouts come pre-transposed from the host: plain row DMAs only
        xall = [xall_pool.tile([P, R, B], BF, tag=f"xall{b}", name=f"xall{b}")
                for b in range(NBLK)]
        for blk in range(NBLK):
            nc.sync.dma_start(
                out=xall[blk][:].rearrange("p r i -> p (r i)"),
                in_=imgT2[P * blk:P * (blk + 1), :])

        # ---------- caption branch ----------
        maskT = smalls.tile([T, CPC], FP, tag="maskT")
        nc.sync.dma_start(out=maskT[:], in_=maskT_d[:, :])
        cap_sb = smalls.tile([CPC, D], FP, tag="cap_sb")
        with tc.tile_pool(name="cap_ps", bufs=2, space="PSUM") as cap_ps_pool:
            for c in range(CPC):
                ct = cap_pool.tile([T, D], FP, tag="cap")
                nc.sync.dma_start(out=ct[:], in_=cap[c, :, :])
                pp = cap_ps_pool.tile([1, D], FP, tag="pp", name="pp")
                for j in range(2):
                    nc.tensor.matmul(pp[:, 512 * j:512 * (j + 1)],
                                     maskT[:, c:c + 1],
                                     ct[:, 512 * j:512 * (j + 1)],
                                     start=True, stop=True,
                                     skip_group_check=True)
                prow = cap_pool.tile([1, D], FP, tag="prow", name="prow",
                                     bufs=2)
                nc.scalar.copy(prow[:], pp[:])
                nc.sync.dma_start(out=cap_sb[c:c + 1, :], in_=prow[:])

        # weight half-0 loads issued early (before the transpose-DMA flood);
        # pool closed manually right after the FiLM matmuls free the space
        w0_cm = tc.tile_pool(name="w0", bufs=1)
        w0_pool = w0_cm.__enter__()
        w0 = {}
        for which, wd in (("g", wgT_d), ("b", wbT_d)):
            for kb in range(NBLK):
                t = w0_pool.tile([P, D // 2], FP, tag=f"w0{which}{kb}",
                                 name=f"w0{which}{kb}")
                nc.sync.dma_start(out=t[:], in_=wd[P * kb:P * (kb + 1), 0:512])
                w0[(which, kb)] = t

        capT = [smalls.tile([P, CPC], FP, tag=f"capT{b}", name=f"capT{b}")
                for b in range(NBLK)]
        capT2 = [smalls.tile([P, CPC, 2], BF, tag=f"capT2{b}",
                             name=f"capT2{b}") for b in range(NBLK)]
        for blk in range(NBLK):
            nc.vector.memset(capT2[blk][:], 1.0)
            pst = tp_psum.tile([P, P], FP, tag="tp")
            nc.tensor.transpose(pst[:, 0:CPC], cap_sb[:, P * blk:P * (blk + 1)],
                                ident[:CPC, :CPC])
            nc.vector.tensor_copy(out=capT[blk][:], in_=pst[:, 0:CPC])
            nc.vector.tensor_copy(out=capT2[blk][:, :, 0], in_=pst[:, 0:CPC])

        scr_c = smalls.tile([CPC, D], FP, tag="scr_c")
        nc.vector.tensor_tensor(out=scr_c[:], in0=cap_sb[:], in1=cap_sb[:],
                                op=Alu.mult)
        n2 = smalls.tile([CPC, 1], FP, tag="n2")
        nc.vector.tensor_reduce(out=n2[:], in_=scr_c[:], axis=Ax.X, op=Alu.add)
        nrm = smalls.tile([CPC, 1], FP, tag="nrm")
        nc.scalar.activation(nrm[:], n2[:], Act.Sqrt)
        nrm_e = smalls.tile([CPC, 1], FP, tag="nrm_e")
        nc.vector.tensor_scalar(out=nrm_e[:], in0=nrm[:], scalar1=L2_EPS,
                                scalar2=None, op0=Alu.add)
        rn = smalls.tile([CPC, 1], FP, tag="rn")
        nc.vector.reciprocal(rn[:], nrm_e[:])

        # ---------- image loads ----------
        sums_sb = smalls.tile([1, D], FP, tag="sums_sb")
        muT = smalls.tile([P, NBLK], FP, tag="muT")
        with tc.tile_pool(name="xt", bufs=1) as xt_pool, \
             tc.tile_pool(name="xs_ps", bufs=1, space="PSUM") as xs_ps_pool:
            xt = [xt_pool.tile([P, D], BF, tag=f"xt{k}", name=f"xt{k}")
                  for k in range(NCH)]
            for k in range(NCH):
                rows = min(P, NIR - P * k)
                nc.sync.dma_start(out=xt[k][0:rows, :],
                                  in_=imgbf[P * k:P * k + rows, :])
            ps = [xs_ps_pool.tile([1, 512], FP, tag=f"xs{h}", name=f"xs{h}")
                  for h in range(2)]
            for h in range(2):
                for k in range(NCH):
                    rows = min(P, NIR - P * k)
                    nc.tensor.matmul(ps[h][:], ones1b[0:rows, :],
                                     xt[k][0:rows, 512 * h:512 * (h + 1)],
                                     start=(k == 0), stop=(k == NCH - 1),
                                     skip_group_check=True)
            for h in range(2):
                nc.scalar.copy(sums_sb[:, 512 * h:512 * (h + 1)], ps[h][:])
        inv_n = 1.0 / float(NIR)
        muT_raw = smalls.tile([P, NBLK], FP, tag="muT_raw")
        for b2 in range(NBLK):
            pst = tp_psum.tile([P, P], FP, tag="tp")
            nc.tensor.transpose(pst[:, 0:1],
                                sums_sb[:, P * b2:P * (b2 + 1)],
                                ident[:1, :1])
            nc.vector.tensor_copy(out=muT_raw[:, b2:b2 + 1], in_=pst[:, 0:1])
        nc.vector.tensor_scalar(out=muT[:], in0=muT_raw[:], scalar1=inv_n,
                                scalar2=None, op0=Alu.mult)

        # x^2 sums (DVE contiguous folds) and X* = max|x| (GpSimd folds)
        m2T = smalls.tile([P, NBLK], FP, tag="m2T")
        xstar = smalls.tile([P, NBLK], FP, tag="xstar")
        for blk in range(NBLK):
            xf = xall[blk][:].rearrange("p r i -> p (r i)")
            x2 = sq_pool.tile([P, NIR], BF, tag="x2")
            nc.scalar.square(x2[:], xf)
            f1 = sq_pool.tile([P, NIR // 2], BF, tag="f1")
            nc.vector.tensor_tensor(out=f1[:], in0=x2[:, 0:NIR // 2],
                                    in1=x2[:, NIR // 2:NIR], op=Alu.add)
            f2 = sq_pool.tile([P, NIR // 4], BF, tag="f2")
            nc.vector.tensor_tensor(out=f2[:], in0=f1[:, 0:NIR // 4],
                                    in1=f1[:, NIR // 4:NIR // 2], op=Alu.add)
            sq_s = sq_pool.tile([P, 1], FP, tag="sq_s", bufs=2)
            nc.vector.tensor_reduce(out=sq_s[:], in_=f2[:], axis=Ax.X,
                                    op=Alu.add)
            nc.vector.tensor_scalar(out=m2T[:, blk:blk + 1], in0=sq_s[:],
                                    scalar1=inv_n, scalar2=None, op0=Alu.mult)
            # X* = max|x| per channel, one absolute-value max-reduce
            nc.vector.tensor_reduce(out=xstar[:, blk:blk + 1], in_=xf,
                                    axis=Ax.X, op=Alu.max,
                                    apply_absolute_value=True)

        # per-channel temperature cap thrS = CLAMP / X*
        rxs = smalls.tile([P, NBLK], FP, tag="rxs")
        nc.vector.reciprocal_approx_fast(out=rxs[:], in_=xstar[:])
        thrS = smalls.tile([P, NBLK], FP, tag="thrS")
        nc.vector.tensor_scalar(out=thrS[:], in0=rxs[:], scalar1=CLAMP,
                                scalar2=None, op0=Alu.mult)
        nthrS = smalls.tile([P, NBLK], FP, tag="nthrS")
        nc.vector.tensor_scalar(out=nthrS[:], in0=thrS[:], scalar1=-1.0,
                                scalar2=None, op0=Alu.mult)

        # BN: rho10 = 10/sqrt(var+eps), mu01 = mu/10
        musqT = smalls.tile([P, NBLK], FP, tag="musqT")
        nc.scalar.square(musqT[:], muT[:])
        varT = smalls.tile([P, NBLK], FP, tag="varT")
        nc.vector.tensor_tensor(out=varT[:], in0=m2T[:], in1=musqT[:],
                                op=Alu.subtract)
        epsT = smalls.tile([P, 1], FP, tag="epsT")
        nc.vector.memset(epsT[:], BN_EPS)
        stdT = smalls.tile([P, NBLK], FP, tag="stdT")
        nc.scalar.activation(stdT[:], varT[:], Act.Sqrt, bias=epsT[:])
        rhoT = smalls.tile([P, NBLK], FP, tag="rhoT")
        nc.vector.reciprocal_approx_fast(out=rhoT[:], in_=stdT[:])
        rho10 = smalls.tile([P, NBLK], FP, tag="rho10")
        nc.vector.tensor_scalar(out=rho10[:], in0=rhoT[:], scalar1=SMOOTH,
                                scalar2=None, op0=Alu.mult)
        mu01 = smalls.tile([P, NBLK], FP, tag="mu01")
        nc.vector.tensor_scalar(out=mu01[:], in0=muT[:], scalar1=1.0 / SMOOTH,
                                scalar2=None, op0=Alu.mult)

        # ---------- FiLM params ----------
        bg1T = smalls.tile([P, NBLK], FP, tag="bg1T")
        nc.sync.dma_start(out=bg1T[:], in_=bg1T_d[:, :])
        bbT = smalls.tile([P, NBLK], FP, tag="bbT")
        nc.sync.dma_start(out=bbT[:], in_=bbT_d[:, :])

        g_all = smalls.tile([P, NBLK, CPC], FP, tag="g_all")
        bta_all = smalls.tile([P, NBLK, CPC], FP, tag="bta_all")
        with tc.tile_pool(name="gb_ps", bufs=4, space="PSUM") as gb_ps_pool:
            for which, wd, dst in (("g", wgT_d, g_all), ("b", wbT_d, bta_all)):
                for half in range(2):
                    ps = gb_ps_pool.tile([CPC, 512], FP, tag="gcd",
                                         name="gcd_ps")
                    for kb in range(NBLK):
                        if half == 0:
                            w = w0[(which, kb)]
                        else:
                            w = w_pool.tile([P, D // 2], FP, tag="w", name="w")
                            nc.sync.dma_start(
                                out=w[:], in_=wd[P * kb:P * (kb + 1),
                                                 512 * half:512 * (half + 1)])
                        nc.tensor.matmul(ps[:], capT[kb][:], w[:],
                                         start=(kb == 0),
                                         stop=(kb == NBLK - 1),
                                         skip_group_check=True)
                    gsb = gcd_pool.tile([CPC, 512], FP, tag="gsb", name="gsb")
                    nc.scalar.copy(gsb[:], ps[:])
                    for j in range(4):
                        db = half * 4 + j
                        pst = tp_psum.tile([P, P], FP, tag="tp")
                        nc.tensor.transpose(pst[:, 0:CPC],
                                            gsb[:, P * j:P * (j + 1)],
                                            ident[:CPC, :CPC])
                        nc.vector.tensor_copy(out=dst[:, db, :],
                                              in_=pst[:, 0:CPC])
        w0_cm.__exit__(None, None, None)

        def bcast_col(t2d):
            return t2d[:].rearrange("p b -> p b ()").broadcast_to(
                [P, NBLK, CPC])

        # a10 = (g + bg1)*rho10 (the raw temperature), aT = a10/10
        a10 = smalls.tile([P, NBLK, CPC], FP, tag="a10")
        t_a = smalls.tile([P, NBLK, CPC], FP, tag="t_a")
        nc.vector.tensor_tensor(out=t_a[:], in0=g_all[:], in1=bcast_col(bg1T),
                                op=Alu.add)
        nc.vector.tensor_tensor(out=a10[:], in0=t_a[:], in1=bcast_col(rho10),
                                op=Alu.mult)
        aT = smalls.tile([P, NBLK, CPC], FP, tag="aT")
        nc.vector.tensor_scalar(out=aT[:], in0=a10[:], scalar1=1.0 / SMOOTH,
                                scalar2=None, op0=Alu.mult)
        # capped temperature s' (used only as the exp scale)
        sc_all = smalls.tile([P, NBLK, CPC], FP, tag="sc_all")
        for blk in range(NBLK):
            nc.vector.tensor_scalar(out=sc_all[:, blk, :],
                                    in0=a10[:, blk, :],
                                    scalar1=thrS[:, blk:blk + 1],
                                    scalar2=nthrS[:, blk:blk + 1],
                                    op0=Alu.min, op1=Alu.max)
        # negb2 = a10*mu01 - (bta + bb)   (u = wa - negb2)
        negb2 = smalls.tile([P, NBLK, CPC], FP, tag="negb2")
        t_b = smalls.tile([P, NBLK, CPC], FP, tag="t_b")
        nc.vector.tensor_tensor(out=t_b[:], in0=bta_all[:],
                                in1=bcast_col(bbT), op=Alu.add)
        t_c2 = smalls.tile([P, NBLK, CPC], FP, tag="t_c2")
        nc.vector.tensor_tensor(out=t_c2[:], in0=a10[:], in1=bcast_col(mu01),
                                op=Alu.mult)
        nc.vector.tensor_tensor(out=negb2[:], in0=t_c2[:], in1=t_b[:],
                                op=Alu.subtract)
        # pre-expanded (over images) copies for the contiguous u-assembly
        aT_exp = smalls.tile([P, NBLK, CPC, B], BF, tag="aT_exp")
        nc.vector.tensor_copy(
            out=aT_exp[:],
            in_=aT[:].rearrange("p b c -> p b c ()").broadcast_to(
                [P, NBLK, CPC, B]))
        nb2_exp = smalls.tile([P, NBLK, CPC, B], BF, tag="nb2_exp")
        nc.vector.tensor_copy(
            out=nb2_exp[:],
            in_=negb2[:].rearrange("p b c -> p b c ()").broadcast_to(
                [P, NBLK, CPC, B]))

        # ---------- main loop ----------
        big_pool = ctx.enter_context(tc.tile_pool(name="big", bufs=3))
        sw_pool = ctx.enter_context(tc.tile_pool(name="sw", bufs=2))
        dots_sb = smalls.tile([CPC, B], FP, tag="dots_sb")
        usq_sb = smalls.tile([CPC, B], FP, tag="usq_sb")
        with tc.tile_pool(name="du_ps", bufs=3, space="PSUM") as du_ps_pool:
            for c in range(CPC):
                # interleaved e/p accumulator halves one gp op per level
                F9 = sw_pool.tile([P, 2, NBLK, 9, B], BF, tag="F9")
                ps_du = du_ps_pool.tile([2, 2 * B], FP, tag="du")
                for blk in range(NBLK):
                    ep = big_pool.tile([P, 2, R, B], BF, tag="ep", bufs=3)
                    nc.scalar.activation(ep[:, 0], xall[blk][:], Act.Exp,
                                         scale=sc_all[:, blk, c:c + 1])
                    nc.vector.tensor_tensor(out=ep[:, 1], in0=ep[:, 0],
                                            in1=xall[blk][:], op=Alu.mult)
                    geng = nc.gpsimd if PFOLD_GPSIMD else nc.vector
                    f18 = big_pool.tile([P, 2, 18, B], BF, tag="f18", bufs=3)
                    nc.vector.tensor_tensor(out=f18[:], in0=ep[:, :, 0:18, :],
                                            in1=ep[:, :, 18:36, :],
                                            op=Alu.add)
                    geng.tensor_tensor(out=F9[:, :, blk],
                                       in0=f18[:, :, 0:9, :],
                                       in1=f18[:, :, 9:18, :], op=Alu.add)
                # batched fold tails: 9 -> (4+4+1) -> 2 -> 1, both chains and
                # all blocks at once, folding in place inside F9
                SWall = sw_pool.tile([P, 2, NBLK, B], FP, tag="SWall")
                nc.vector.tensor_tensor(out=F9[:, :, :, 0:4, :],
                                        in0=F9[:, :, :, 0:4, :],
                                        in1=F9[:, :, :, 4:8, :], op=Alu.add)
                nc.vector.tensor_tensor(out=F9[:, :, :, 0:2, :],
                                        in0=F9[:, :, :, 0:2, :],
                                        in1=F9[:, :, :, 2:4, :], op=Alu.add)
                nc.vector.tensor_tensor(out=F9[:, :, :, 0, :],
                                        in0=F9[:, :, :, 0, :],
                                        in1=F9[:, :, :, 1, :], op=Alu.add)
                nc.vector.tensor_tensor(out=SWall[:], in0=F9[:, :, :, 0, :],
                                        in1=F9[:, :, :, 8, :], op=Alu.add)
                rs = sw_pool.tile([P, NBLK, B], FP, tag="rs")
                nc.vector.reciprocal_approx_fast(
                    out=rs[:].rearrange("p b i -> p (b i)"),
                    in_=SWall[:, 0].rearrange("p b i -> p (b i)"))
                t1 = sw_pool.tile([P, NBLK, B], BF, tag="t1")
                nc.vector.tensor_tensor(out=t1[:], in0=SWall[:, 1], in1=rs[:],
                                        op=Alu.mult)
                uu_big = sw_pool.tile([P, 2, NBLK, B], BF, tag="uu")
                t2 = sw_pool.tile([P, NBLK, B], BF, tag="t2")
                nc.vector.tensor_tensor(out=t2[:], in0=t1[:],
                                        in1=aT_exp[:, :, c, :], op=Alu.mult)
                nc.vector.tensor_tensor(out=uu_big[:, 0], in0=t2[:],
                                        in1=nb2_exp[:, :, c, :],
                                        op=Alu.subtract)
                nc.vector.tensor_tensor(out=uu_big[:, 1], in0=uu_big[:, 0],
                                        in1=uu_big[:, 0], op=Alu.mult)
                for blk in range(NBLK):
                    nc.tensor.matmul(ps_du[:], capT2[blk][:, c, :],
                                     uu_big[:, :, blk, :],
                                     start=(blk == 0), stop=(blk == NBLK - 1),
                                     skip_group_check=True)
                du_c = sw_pool.tile([2, 2 * B], FP, tag="du_c", bufs=2)
                nc.scalar.copy(du_c[:], ps_du[:])
                nc.sync.dma_start(out=dots_sb[c:c + 1, :],
                                  in_=du_c[0:1, 0:B])
                nc.sync.dma_start(out=usq_sb[c:c + 1, :],
                                  in_=du_c[1:2, B:2 * B])

        # ---------- tail ----------
        sq = smalls.tile([CPC, B], FP, tag="sqf")
        nc.scalar.activation(sq[:], usq_sb[:], Act.Sqrt)
        ru = smalls.tile([CPC, B], FP, tag="ruf")
        nc.vector.reciprocal_approx_fast(out=ru[:], in_=sq[:])
        t3 = smalls.tile([CPC, B], FP, tag="t3f")
        nc.vector.tensor_tensor(out=t3[:], in0=dots_sb[:], in1=ru[:],
                                op=Alu.mult)
        out_sb = smalls.tile([CPC, B], FP, tag="out_sb")
        nc.vector.tensor_scalar(out=out_sb[:], in0=t3[:], scalar1=rn[:, 0:1],
                                scalar2=None, op0=Alu.mult)
        nc.sync.dma_start(out=out_d[:, :], in_=out_sb[:])

    nc.compile()
    return nc


def _get_nc():
    if "nc" not in _CACHE:
        _CACHE["nc"] = _build_nc()
    return _CACHE["nc"]


def kernel(img_embed, cap_embed, lens, W_gamma, b_gamma, W_beta, b_beta,
           _want_trace=False):
    from concourse.bass_utils import run_bass_kernel_spmd

    nc = _get_nc()

    img_embed = np.asarray(img_embed, np.float32)
    cap_embed = np.asarray(cap_embed, np.float32)
    lens_np = np.asarray(lens)
    W_gamma = np.asarray(W_gamma, np.float32)
    W_beta = np.asarray(W_beta, np.float32)
    b_gamma = np.asarray(b_gamma, np.float32)
    b_beta = np.asarray(b_beta, np.float32)

    import ml_dtypes
    # rows (r, i)-major so the on-chip tiles are [P, R, B]
    img_ri = img_embed.transpose(1, 0, 2).reshape(NIR, D)
    img_bf = np.ascontiguousarray(img_ri.astype(ml_dtypes.bfloat16))
    img_t2 = np.ascontiguousarray(img_bf.T)
    wgT = np.ascontiguousarray(W_gamma.T)
    wbT = np.ascontiguousarray(W_beta.T)
    bg1T = np.ascontiguousarray((1.0 + b_gamma).reshape(NBLK, P).T)
    bbT = np.ascontiguousarray(b_beta.reshape(NBLK, P).T)

    lens_f = lens_np.astype(np.float64)
    mask = (np.arange(T)[None, :] < lens_np[:, None]).astype(np.float64)
    mask = (mask / lens_f[:, None]).astype(np.float32)  # (B, T)

    in_maps = []
    for k in range(NCORES):
        sl = slice(k * CPC, (k + 1) * CPC)
        in_maps.append({
            "imgbf": img_bf,
            "imgT2": img_t2,
            "cap": np.ascontiguousarray(cap_embed[sl]),
            "maskT": np.ascontiguousarray(mask[sl].T),
            "wgT": wgT,
            "wbT": wbT,
            "bg1T": bg1T,
            "bbT": bbT,
        })

    kw = {}
    if _want_trace:
        import os as _os2, shutil as _sh
        _sh.rmtree("/tmp/ktrace", ignore_errors=True)
        _os2.makedirs("/tmp/ktrace", exist_ok=True)
        kw = {"tmpdir": "/tmp/ktrace"}
    res = run_bass_kernel_spmd(nc, in_maps, core_ids=list(range(NCORES)),
                               trace=_want_trace, **kw)
    outs = [np.asarray(r["out"]) for r in res.results]
    sims = np.concatenate([o.T for o in outs], axis=1).astype(np.float32)
    if _want_trace:
        return sims, res
    return sims
